# revision 1
# baseline (speedup 1.0000x reference)
"""Trainium2 Bass kernel for nn_CropperQAT (multi-scale RoIAlign with
fake-quantized rois) — v2.

Strategy (data-parallel over (roi, scale) jobs, 8 cores):
  * Host replicates the reference roi math bit-exactly (numpy f32), then for
    each job (roi a, scale s) derives a 9-row x 10-px fp16 feature window
    (all bilinear taps fit: roi h/w <= 8) plus interpolation weights.
  * Features are stored channels-last fp16 as 2-px "units" (256B) so
    dma_gather (int16 indices, 256B-stride) can fetch windows; the unit
    space is split into 6 overlapping regions of 30000 units so relative
    indices fit int16.
  * Device, x-regular jobs (partition = (job k, window row e), 14x9=126):
      - dma_gather batches up to 8 groups: [128, G*640] fp16
      - x-interp: ACT copy-scale + DVE scalar_tensor_tensor (2 taps,
        per-job scalar weights; groups are single-parity so the slice
        offset is group-constant)
      - y-interp: PE matmul with per-group block-diagonal [126->112]
        fp16 weights (handles y clamping/validity for free) -> PSUM f32
      - ACT copy PSUM -> fp16 output tile; batched DMA out.
  * x-irregular jobs (~6%): full bilinear as two accumulating matmuls
    (even/odd pixel parity) per 2-job pair slot; zero DVE work.
  * Host converts fp16 device output to f32 (tolerance is generous; fp16
    path measures ~5e-4 rel err).
"""
import os
import sys

sys.path.insert(0, "/opt/trn_rl_repo")

import numpy as np

import concourse.bass as bass
import concourse.bacc as bacc
import concourse.mybir as mybir
from concourse.tile import TileContext
from concourse.bass_utils import run_bass_kernel_spmd

F32 = np.float32
SIZE = 8
STRIDES = (4, 8, 16)
QS = np.float32(0.25)
C = 64
N_CORES = 8
JOBS_PG = 14                 # jobs per regular group (x9 rows = 126 partitions)
ROWS = 9                     # y window rows
PXW = 10                     # x window pixels (5 units)
UNIT = 128                   # fp16 elems per unit (2 px * 64 ch)
ESIZE = PXW * C              # 640 fp16 per gathered row-window element
REGION = 30000               # units per region (int16 headroom: +1028 < 32768)
NGMAX = 8                    # groups per dma_gather batch
NSMAX = 8                    # irr pair-slots per dma_gather batch

LAST_RESULTS = None


# ----------------------------------------------------------------------------
# host-side math (bit-exact replication of the jax reference)
# ----------------------------------------------------------------------------

def _fake_quant(x):
    return (np.clip(np.round(x / QS), -32768, 32767) * QS).astype(F32)


def _prep(c, L):
    valid = (c >= -1.0) & (c <= L)
    c = np.maximum(c, F32(0.0))
    low0 = np.floor(c).astype(np.int32)
    hi_edge = low0 >= L - 1
    low = np.where(hi_edge, L - 1, low0).astype(np.int32)
    high = np.where(hi_edge, L - 1, low0 + 1).astype(np.int32)
    c = np.where(hi_edge, F32(L - 1), c).astype(F32)
    frac = (c - low.astype(F32)).astype(F32)
    return low, high, frac, valid


def _scale_tables(pixel, batch_index, stride, H, W, base_px):
    """Per-job tables for one scale. base_px = pixel offset of this scale's
    image block in the channels-last concatenated feature tensor."""
    A = pixel.shape[0]
    st = F32(stride)
    half = F32(SIZE / 2.0)
    centers = (np.arange(SIZE, dtype=F32) + F32(0.5)).astype(F32)

    px = pixel[:, 0].astype(F32)
    py = pixel[:, 1].astype(F32)
    x1 = _fake_quant(np.maximum(px / st - half, F32(0.0)).astype(F32))
    y1 = _fake_quant(np.maximum(py / st - half, F32(0.0)).astype(F32))
    x2 = _fake_quant(np.maximum(px / st + half, F32(0.0)).astype(F32))
    y2 = _fake_quant(np.maximum(py / st + half, F32(0.0)).astype(F32))
    roi_w = np.maximum(x2 - x1, F32(1.0)).astype(F32)
    roi_h = np.maximum(y2 - y1, F32(1.0)).astype(F32)
    y = (y1[:, None] + centers[None, :] * (roi_h / F32(SIZE))[:, None]).astype(F32)
    x = (x1[:, None] + centers[None, :] * (roi_w / F32(SIZE))[:, None]).astype(F32)

    yl, yh, fy, vy = _prep(y, H)
    xl, xh, fx, vx = _prep(x, W)

    b = batch_index.astype(np.int64)

    # y window + Wy (y-interp matrix with validity folded in)
    wy0 = np.minimum(yl[:, 0], H - ROWS).astype(np.int64)
    ey_lo = yl.astype(np.int64) - wy0[:, None]
    ey_hi = yh.astype(np.int64) - wy0[:, None]
    assert ey_lo.min() >= 0 and ey_lo.max() <= 8
    assert ey_hi.min() >= 0 and ey_hi.max() <= 8
    vyf = vy.astype(F32)
    wl = ((F32(1.0) - fy) * vyf).astype(F32)
    wh = (fy * vyf).astype(F32)
    Wy = np.zeros((A, ROWS, SIZE), F32)
    aa = np.repeat(np.arange(A), SIZE)
    ii = np.tile(np.arange(SIZE), A)
    np.add.at(Wy, (aa, ey_lo.ravel(), ii), wl.ravel())
    np.add.at(Wy, (aa, ey_hi.ravel(), ii), wh.ravel())

    # x window
    ar = np.arange(SIZE, dtype=np.int32)
    reg = (np.all(xl == xl[:, :1] + ar[None, :], axis=1)
           & np.all(xh == xl + 1, axis=1)
           & np.all(vx, axis=1)
           & np.all(fx == fx[:, :1], axis=1))
    xmin = np.minimum(xl.min(axis=1), xh.min(axis=1)).astype(np.int64)
    x0w = np.clip(xmin & ~1, 0, W - PXW)
    q0 = (xmin - x0w).astype(np.int64)          # parity offset for reg jobs
    ex_lo = xl.astype(np.int64) - x0w[:, None]
    ex_hi = xh.astype(np.int64) - x0w[:, None]
    assert ex_lo.min() >= 0 and ex_lo.max() <= PXW - 1
    assert ex_hi.min() >= 0 and ex_hi.max() <= PXW - 1
    assert np.all(q0[reg] <= 1)
    bx0 = (F32(1.0) - fx[:, 0]).astype(F32)
    bx1 = fx[:, 0].astype(F32)

    # irr: dense x matrix over the 10-px window (validity folded in)
    vxf = vx.astype(F32)
    Mx = np.zeros((A, PXW, SIZE), F32)
    jj = np.tile(np.arange(SIZE), A)
    np.add.at(Mx, (aa, ex_lo.ravel(), jj), ((F32(1.0) - fx) * vxf).ravel())
    np.add.at(Mx, (aa, ex_hi.ravel(), jj), (fx * vxf).ravel())

    # window start unit per row e: u0 + e*(W//2)
    u0 = (base_px + (b * H + wy0) * W + x0w) // 2
    return dict(Wy=Wy, reg=reg, q0=q0, bx0=bx0, bx1=bx1, Mx=Mx,
                u0=u0.astype(np.int64), wrow=W // 2)


def _host_prep(f0, f1, f2, pixel, batch_index):
    A = pixel.shape[0]
    feats = (f0, f1, f2)

    cat = np.concatenate([
        np.ascontiguousarray(np.asarray(f, dtype=F32).transpose(0, 2, 3, 1))
        .reshape(-1, C) for f in feats], axis=0).astype(np.float16)
    nunits = cat.shape[0] // 2
    nreg = (nunits - 1) // REGION + 1
    nunits_pad = (nreg - 1) * REGION + 32768 + 8
    cat2 = np.zeros((nunits_pad, UNIT), np.float16)
    cat2[:nunits] = cat.reshape(nunits, UNIT)

    tabs = []
    base_px = 0
    for s, f in enumerate(feats):
        H, W = f.shape[2], f.shape[3]
        tabs.append(_scale_tables(np.asarray(pixel, F32),
                                  np.asarray(batch_index), STRIDES[s],
                                  H, W, base_px))
        base_px += 4 * H * W

    # ---- flat job lists with stratum key (region, kind)
    jobs = []            # (s, a, region, parity, is_reg)
    for s in range(3):
        t = tabs[s]
        regions = t["u0"] // REGION
        for a in range(A):
            jobs.append((s, a, int(regions[a]), int(t["q0"][a]),
                         bool(t["reg"][a])))

    # stratified round-robin assignment:
    #   reg strata key (region, parity), irr strata key (region)
    core_reg = {}        # (region, parity) -> [list per core of (s,a)]
    core_irr = {}        # region -> [list per core]
    cnt_reg = {}
    cnt_irr = {}
    for (s, a, r, q, isreg) in jobs:
        if isreg:
            key = (r, q)
            lst = core_reg.setdefault(key, [[] for _ in range(N_CORES)])
            k = cnt_reg[key] = cnt_reg.get(key, 0) + 1
            lst[(k - 1) % N_CORES].append((s, a))
        else:
            lst = core_irr.setdefault(r, [[] for _ in range(N_CORES)])
            k = cnt_irr[r] = cnt_irr.get(r, 0) + 1
            lst[(k - 1) % N_CORES].append((s, a))

    # global group counts per stratum (max over cores)
    reg_strata = sorted(core_reg.keys())
    irr_strata = sorted(core_irr.keys())
    NB = {key: max(-(-len(lst) // JOBS_PG) for lst in core_reg[key])
          for key in reg_strata}
    NP_ = {r: max(-(-len(lst) // 2) for lst in core_irr[r])
           for r in irr_strata}
    NRGtot = sum(NB.values())
    NPtot = sum(NP_.values())

    # batches (static program structure)
    reg_batches = []     # (region, parity, n_groups_in_batch, gi0)
    gi = 0
    for (r, q) in reg_strata:
        nb = NB[(r, q)]
        while nb > 0:
            g = min(nb, NGMAX)
            reg_batches.append((r, q, g, gi))
            gi += g
            nb -= g
    irr_batches = []     # (region, n_slots_in_batch, pi0)
    pi = 0
    for r in irr_strata:
        npr = NP_[r]
        while npr > 0:
            sct = min(npr, NSMAX)
            irr_batches.append((r, sct, pi))
            pi += sct
            npr -= sct

    # ---- per-core packed device inputs + output mapping
    RCOLS = sum(8 * g for (_, _, g, _) in reg_batches)
    ICOLS = sum(8 * sct for (_, sct, _) in irr_batches)
    per_core = []
    for ccc in range(N_CORES):
        ridx = np.zeros((128, max(RCOLS, 1)), np.int16)
        rw = np.zeros((128, max(NRGtot, 1) * 112), np.float16)
        rs = np.zeros((128, max(NRGtot, 1) * 2), F32)
        iidx = np.zeros((128, max(ICOLS, 1)), np.int16)
        iw = np.zeros((128, max(NPtot, 1) * 256), np.float16)
        rmap = []        # (gi, k, s, a)
        imap = []        # (pi, k2, s, a)

        col0 = 0
        for (r, q, G, gi0) in reg_batches:
            lst = core_reg[(r, q)][ccc]
            idxvals = np.zeros((G * 128,), np.int64)
            sg0 = _stratum_gi0(reg_batches, r, q)
            for g in range(G):
                for k in range(JOBS_PG):
                    j = (gi0 - sg0 + g) * JOBS_PG + k
                    if j < len(lst):
                        s, a = lst[j]
                        t = tabs[s]
                        gidx = gi0 + g
                        rmap.append((gidx, k, s, a))
                        rel0 = int(t["u0"][a] - r * REGION)
                        for e in range(ROWS):
                            idxvals[g * 128 + k * ROWS + e] = rel0 + e * t["wrow"]
                        rw[k * ROWS:(k + 1) * ROWS,
                           gidx * 112 + k * SIZE: gidx * 112 + (k + 1) * SIZE] = \
                            t["Wy"][a].astype(np.float16)
                        rs[k * ROWS:(k + 1) * ROWS, gidx * 2] = t["bx0"][a]
                        rs[k * ROWS:(k + 1) * ROWS, gidx * 2 + 1] = t["bx1"][a]
            assert idxvals.max() < 32768
            cols = G * 8
            tilecols = _wrap_idx(idxvals)
            ridx[:, col0: col0 + cols] = tilecols
            col0 += cols

        col0 = 0
        for (r, S, pi0) in irr_batches:
            lst = core_irr[r][ccc]
            idxvals = np.zeros((S * 128,), np.int64)
            for sl in range(S):
                for k2 in range(2):
                    j = (pi0 - _irr_pi0(irr_batches, r) + sl) * 2 + k2
                    if j < len(lst):
                        s, a = lst[j]
                        t = tabs[s]
                        pidx = pi0 + sl
                        imap.append((pidx, k2, s, a))
                        rel0 = int(t["u0"][a] - r * REGION)
                        for e in range(ROWS):
                            for u in range(PXW // 2):
                                idxvals[sl * 128 + k2 * 45 + e * 5 + u] = \
                                    rel0 + e * t["wrow"] + u
                        # W2 par matrices [45, 64] each
                        Wy = t["Wy"][a]                     # [9, 8]
                        Mx = t["Mx"][a]                     # [10, 8]
                        for par in range(2):
                            Mp = Mx[par::2]                 # [5, 8]
                            W2 = np.einsum("ei,uj->euij", Wy, Mp).reshape(45, 64)
                            iw[k2 * 45:(k2 + 1) * 45,
                               pidx * 256 + par * 128 + k2 * 64:
                               pidx * 256 + par * 128 + (k2 + 1) * 64] = \
                                W2.astype(np.float16)
            assert idxvals.max() < 32768
            cols = S * 8
            iidx[:, col0: col0 + cols] = _wrap_idx(idxvals)
            col0 += cols

        per_core.append(dict(ridx=ridx, rw=rw, rs=rs, iidx=iidx, iw=iw,
                             rmap=rmap, imap=imap))

    return dict(cat2=cat2, per_core=per_core, tabs=tabs,
                reg_batches=reg_batches, irr_batches=irr_batches,
                NRGtot=NRGtot, NPtot=NPtot, RCOLS=RCOLS, ICOLS=ICOLS,
                nunits_pad=nunits_pad, A=A)


def _stratum_gi0(reg_batches, r, q):
    for (rr, qq, G, gi0) in reg_batches:
        if (rr, qq) == (r, q):
            return gi0
    raise KeyError


def _irr_pi0(irr_batches, r):
    for (rr, S, pi0) in irr_batches:
        if rr == r:
            return pi0
    raise KeyError


def _wrap_idx(idxvals):
    """[N] linear idx values -> [128, N//16] int16 tile (16-partition wrap,
    replicated to all 128 partitions)."""
    n = len(idxvals)
    assert n % 16 == 0
    tile = np.zeros((128, n // 16), np.int16)
    lin = np.asarray(idxvals, np.int64)
    assert lin.min() >= 0 and lin.max() < 32768
    t16 = lin.reshape(n // 16, 16).T.astype(np.int16)   # [16, n/16]
    for rep in range(8):
        tile[rep * 16:(rep + 1) * 16, :] = t16
    return tile


# ----------------------------------------------------------------------------
# device program
# ----------------------------------------------------------------------------

def _build_program(prep):
    f16 = mybir.dt.float16
    f32 = mybir.dt.float32
    i16 = mybir.dt.int16
    COPY = mybir.ActivationFunctionType.Copy
    MULT = mybir.AluOpType.mult
    ADD = mybir.AluOpType.add
    NRGtot = prep["NRGtot"]
    NPtot = prep["NPtot"]
    RCOLS = prep["RCOLS"]
    ICOLS = prep["ICOLS"]
    NUP = prep["nunits_pad"]

    nc = bacc.Bacc("TRN2")
    cat2_t = nc.dram_tensor("cat2", [NUP, UNIT], f16, kind="ExternalInput")
    ridx_t = nc.dram_tensor("ridx", [128, max(RCOLS, 1)], i16,
                            kind="ExternalInput")
    rw_t = nc.dram_tensor("rw", [128, max(NRGtot, 1) * 112], f16,
                          kind="ExternalInput")
    rs_t = nc.dram_tensor("rs", [128, max(NRGtot, 1) * 2], f32,
                          kind="ExternalInput")
    iidx_t = nc.dram_tensor("iidx", [128, max(ICOLS, 1)], i16,
                            kind="ExternalInput")
    iw_t = nc.dram_tensor("iw", [128, max(NPtot, 1) * 256], f16,
                          kind="ExternalInput")
    oreg_t = nc.dram_tensor("out_reg", [128, max(NRGtot, 1) * 512], f16,
                            kind="ExternalOutput")
    oirr_t = nc.dram_tensor("out_irr", [128, max(NPtot, 1) * 64], f16,
                            kind="ExternalOutput")

    with TileContext(nc) as tc:
        with tc.tile_pool(name="const", bufs=1) as cpool, \
             tc.tile_pool(name="gat", bufs=3) as gpool, \
             tc.tile_pool(name="mv", bufs=8) as vpool, \
             tc.tile_pool(name="ps", bufs=4, space="PSUM") as pspool, \
             tc.tile_pool(name="ob", bufs=3) as obpool:

            ridx = cpool.tile([128, max(RCOLS, 1)], i16)
            rw = cpool.tile([128, max(NRGtot, 1) * 112], f16)
            rs = cpool.tile([128, max(NRGtot, 1) * 2], f32)
            iidx = cpool.tile([128, max(ICOLS, 1)], i16)
            iw = cpool.tile([128, max(NPtot, 1) * 256], f16)
            nc.sync.dma_start(ridx[:, :], ridx_t[:, :])
            nc.sync.dma_start(rw[:, :], rw_t[:, :])
            nc.sync.dma_start(rs[:, :], rs_t[:, :])
            nc.sync.dma_start(iidx[:, :], iidx_t[:, :])
            nc.sync.dma_start(iw[:, :], iw_t[:, :])

            def region_ap(r, esize):
                return bass.AP(cat2_t, r * REGION * UNIT,
                               [(UNIT, 32768), (1, esize)])

            col0 = 0
            for (r, q, G, gi0) in prep["reg_batches"]:
                Gt = gpool.tile([128, G * ESIZE], f16, tag="gt")
                nc.gpsimd.dma_gather(
                    out_ap=Gt[:, :].rearrange("p (s e) -> p s e", e=ESIZE),
                    in_ap=region_ap(r, ESIZE),
                    idxs_ap=ridx[:, col0: col0 + G * 8],
                    num_idxs=G * 128, num_idxs_reg=G * 128,
                    elem_size=ESIZE, elem_step=UNIT)
                col0 += G * 8
                obuf = obpool.tile([128, G * 512], f16, tag="ob")
                for g in range(G):
                    gi = gi0 + g
                    base = g * ESIZE + 64 * q
                    m = vpool.tile([128, 512], f16, tag="m")
                    nc.scalar.activation(m[:, :], Gt[:, base: base + 512],
                                         COPY, scale=rs[:, 2 * gi: 2 * gi + 1])
                    v = vpool.tile([128, 512], f16, tag="v")
                    nc.vector.scalar_tensor_tensor(
                        out=v[:, :], in0=Gt[:, base + 64: base + 576],
                        scalar=rs[:, 2 * gi + 1: 2 * gi + 2], in1=m[:, :],
                        op0=MULT, op1=ADD)
                    u = pspool.tile([112, 512], f32, tag="u")
                    nc.tensor.matmul(out=u[:, :],
                                     lhsT=rw[:, gi * 112: (gi + 1) * 112],
                                     rhs=v[:, :], start=True, stop=True)
                    nc.scalar.activation(obuf[:112, g * 512: (g + 1) * 512],
                                         u[:, :], COPY)
                nc.sync.dma_start(oreg_t[:, gi0 * 512: (gi0 + G) * 512],
                                  obuf[:, :])

            col0 = 0
            for (r, S, pi0) in prep["irr_batches"]:
                Git = gpool.tile([128, S * UNIT], f16, tag="git")
                nc.gpsimd.dma_gather(
                    out_ap=Git[:, :].rearrange("p (s e) -> p s e", e=UNIT),
                    in_ap=region_ap(r, UNIT),
                    idxs_ap=iidx[:, col0: col0 + S * 8],
                    num_idxs=S * 128, num_idxs_reg=S * 128,
                    elem_size=UNIT, elem_step=UNIT)
                col0 += S * 8
                iob = obpool.tile([128, S * 64], f16, tag="iob")
                for sl in range(S):
                    pi = pi0 + sl
                    u2 = pspool.tile([128, 64], f32, tag="u2")
                    nc.tensor.matmul(
                        out=u2[:, :],
                        lhsT=iw[:, pi * 256: pi * 256 + 128],
                        rhs=Git[:, sl * UNIT: sl * UNIT + 64],
                        start=True, stop=False)
                    nc.tensor.matmul(
                        out=u2[:, :],
                        lhsT=iw[:, pi * 256 + 128: pi * 256 + 256],
                        rhs=Git[:, sl * UNIT + 64: sl * UNIT + 128],
                        start=False, stop=True)
                    nc.vector.tensor_copy(out=iob[:, sl * 64: (sl + 1) * 64],
                                          in_=u2[:, :])
                nc.sync.dma_start(oirr_t[:, pi0 * 64: (pi0 + S) * 64],
                                  iob[:, :])

    nc.finalize()
    return nc


# ----------------------------------------------------------------------------
# entry point
# ----------------------------------------------------------------------------

def kernel(f0, f1, f2, pixel, batch_index):
    global LAST_RESULTS
    prep = _host_prep(f0, f1, f2, pixel, batch_index)
    A = prep["A"]

    nc = _build_program(prep)

    in_maps = []
    for ccc in range(N_CORES):
        pc = prep["per_core"][ccc]
        in_maps.append({"cat2": prep["cat2"], "ridx": pc["ridx"],
                        "rw": pc["rw"], "rs": pc["rs"],
                        "iidx": pc["iidx"], "iw": pc["iw"]})

    res = run_bass_kernel_spmd(nc, in_maps, core_ids=list(range(N_CORES)),
                               trace=bool(os.environ.get("BASS_TRACE")))
    LAST_RESULTS = res

    out = np.zeros((A, 3, C, SIZE, SIZE), F32)
    NRGtot, NPtot = prep["NRGtot"], prep["NPtot"]
    for ccc in range(N_CORES):
        pc = prep["per_core"][ccc]
        raw = res.results[ccc]["out_reg"].astype(F32)
        # [128, NRG*512] -> [14jobs, 8i, NRG, 8j, 64c] -> [14, NRG, 64, 8, 8]
        rr = (raw[:112].reshape(JOBS_PG, SIZE, NRGtot, SIZE, C)
              .transpose(0, 2, 4, 1, 3))
        if pc["rmap"]:
            gia = np.array([m[0] for m in pc["rmap"]])
            ka = np.array([m[1] for m in pc["rmap"]])
            sa = np.array([m[2] for m in pc["rmap"]])
            aa = np.array([m[3] for m in pc["rmap"]])
            out[aa, sa] = rr[ka, gia]
        if pc["imap"]:
            rawi = res.results[ccc]["out_irr"].astype(F32)
            # [128, NP*64]: part = k2*64 + i*8 + j -> [2, 8i, 8j, NP, 64c]
            ri = (rawi.reshape(2, SIZE, SIZE, NPtot, C)
                  .transpose(0, 3, 4, 1, 2))
            pia = np.array([m[0] for m in pc["imap"]])
            k2a = np.array([m[1] for m in pc["imap"]])
            sa = np.array([m[2] for m in pc["imap"]])
            aa = np.array([m[3] for m in pc["imap"]])
            out[aa, sa] = ri[k2a, pia]
    return out.reshape(A, 3 * C, SIZE, SIZE)


# ----------------------------------------------------------------------------
# numpy emulation of the device program (for offline validation)
# ----------------------------------------------------------------------------

def emulate(f0, f1, f2, pixel, batch_index):
    prep = _host_prep(f0, f1, f2, pixel, batch_index)
    A = prep["A"]
    cat2 = prep["cat2"]
    flat = cat2.reshape(-1)
    NRGtot, NPtot = prep["NRGtot"], prep["NPtot"]
    out = np.zeros((A, 3, C, SIZE, SIZE), F32)
    for ccc in range(N_CORES):
        pc = prep["per_core"][ccc]
        raw = np.zeros((128, max(NRGtot, 1) * 512), np.float16)
        col0 = 0
        for (r, q, G, gi0) in prep["reg_batches"]:
            # gather
            Gt = np.zeros((128, G * ESIZE), np.float16)
            for i in range(G * 128):
                p, sslot = i % 128, i // 128
                idx = int(pc["ridx"][i % 16, col0 + i // 16])
                st = (r * REGION + idx) * UNIT
                Gt[p, sslot * ESIZE: (sslot + 1) * ESIZE] = flat[st: st + ESIZE]
            col0 += G * 8
            for g in range(G):
                gi = gi0 + g
                base = g * ESIZE + 64 * q
                g32 = Gt.astype(F32)
                m = (g32[:, base: base + 512]
                     * pc["rs"][:, 2 * gi: 2 * gi + 1]).astype(np.float16)
                v = (g32[:, base + 64: base + 576]
                     * pc["rs"][:, 2 * gi + 1: 2 * gi + 2]
                     + m.astype(F32)).astype(np.float16)
                u = (pc["rw"][:, gi * 112: (gi + 1) * 112].astype(F32).T
                     @ v.astype(F32))
                raw[:112, gi * 512: (gi + 1) * 512] = u.astype(np.float16)
        rr = (raw[:112].astype(F32).reshape(JOBS_PG, SIZE, NRGtot, SIZE, C)
              .transpose(0, 2, 4, 1, 3))
        if pc["rmap"]:
            gia = np.array([m[0] for m in pc["rmap"]])
            ka = np.array([m[1] for m in pc["rmap"]])
            sa = np.array([m[2] for m in pc["rmap"]])
            aa = np.array([m[3] for m in pc["rmap"]])
            out[aa, sa] = rr[ka, gia]

        rawi = np.zeros((128, max(NPtot, 1) * 64), np.float16)
        col0 = 0
        for (r, S, pi0) in prep["irr_batches"]:
            Git = np.zeros((128, S * UNIT), np.float16)
            for i in range(S * 128):
                p, sslot = i % 128, i // 128
                idx = int(pc["iidx"][i % 16, col0 + i // 16])
                st = (r * REGION + idx) * UNIT
                Git[p, sslot * UNIT: (sslot + 1) * UNIT] = flat[st: st + UNIT]
            col0 += S * 8
            for sl in range(S):
                pi = pi0 + sl
                u2 = (pc["iw"][:, pi * 256: pi * 256 + 128].astype(F32).T
                      @ Git[:, sl * UNIT: sl * UNIT + 64].astype(F32))
                u2 += (pc["iw"][:, pi * 256 + 128: pi * 256 + 256]
                       .astype(F32).T
                       @ Git[:, sl * UNIT + 64: sl * UNIT + 128].astype(F32))
                rawi[:, pi * 64: (pi + 1) * 64] = u2.astype(np.float16)
        if pc["imap"]:
            ri = (rawi.astype(F32).reshape(2, SIZE, SIZE, NPtot, C)
                  .transpose(0, 3, 4, 1, 2))
            pia = np.array([m[0] for m in pc["imap"]])
            k2a = np.array([m[1] for m in pc["imap"]])
            sa = np.array([m[2] for m in pc["imap"]])
            aa = np.array([m[3] for m in pc["imap"]])
            out[aa, sa] = ri[k2a, pia]
    return out.reshape(A, 3 * C, SIZE, SIZE)



# revision 4
# speedup vs baseline: 1.0882x; 1.0882x over previous
"""Trainium2 Bass kernel for nn_CropperQAT (multi-scale RoIAlign with
fake-quantized rois) — v2.

Strategy (data-parallel over (roi, scale) jobs, 8 cores):
  * Host replicates the reference roi math bit-exactly (numpy f32), then for
    each job (roi a, scale s) derives a 9-row x 10-px fp16 feature window
    (all bilinear taps fit: roi h/w <= 8) plus interpolation weights.
  * Features are stored channels-last fp16 as 2-px "units" (256B) so
    dma_gather (int16 indices, 256B-stride) can fetch windows; the unit
    space is split into 6 overlapping regions of 30000 units so relative
    indices fit int16.
  * Device, x-regular jobs (partition = (job k, window row e), 14x9=126):
      - dma_gather batches up to 8 groups: [128, G*640] fp16
      - x-interp: ACT copy-scale + DVE scalar_tensor_tensor (2 taps,
        per-job scalar weights; groups are single-parity so the slice
        offset is group-constant)
      - y-interp: PE matmul with per-group block-diagonal [126->112]
        fp16 weights (handles y clamping/validity for free) -> PSUM f32
      - ACT copy PSUM -> fp16 output tile; batched DMA out.
  * x-irregular jobs (~6%): full bilinear as two accumulating matmuls
    (even/odd pixel parity) per 2-job pair slot; zero DVE work.
  * Host converts fp16 device output to f32 (tolerance is generous; fp16
    path measures ~5e-4 rel err).
"""
import os
import sys

sys.path.insert(0, "/opt/trn_rl_repo")

import numpy as np

import concourse.bass as bass
import concourse.bacc as bacc
import concourse.mybir as mybir
from concourse.tile import TileContext
from concourse.bass_utils import run_bass_kernel_spmd

F32 = np.float32
SIZE = 8
STRIDES = (4, 8, 16)
QS = np.float32(0.25)
C = 64
N_CORES = 8
JOBS_PG = 14                 # jobs per regular group (x9 rows = 126 partitions)
ROWS = 9                     # y window rows
PXW = 10                     # x window pixels (5 units)
UNIT = 128                   # fp16 elems per unit (2 px * 64 ch)
ESIZE = PXW * C              # 640 fp16 per gathered row-window element
REGION = 30000               # units per region (int16 headroom: +1028 < 32768)
NGMAX = 8                    # groups per dma_gather batch
NSMAX = 8                    # irr pair-slots per dma_gather batch

LAST_RESULTS = None


# ----------------------------------------------------------------------------
# host-side math (bit-exact replication of the jax reference)
# ----------------------------------------------------------------------------

def _fake_quant(x):
    return (np.clip(np.round(x / QS), -32768, 32767) * QS).astype(F32)


def _prep(c, L):
    valid = (c >= -1.0) & (c <= L)
    c = np.maximum(c, F32(0.0))
    low0 = np.floor(c).astype(np.int32)
    hi_edge = low0 >= L - 1
    low = np.where(hi_edge, L - 1, low0).astype(np.int32)
    high = np.where(hi_edge, L - 1, low0 + 1).astype(np.int32)
    c = np.where(hi_edge, F32(L - 1), c).astype(F32)
    frac = (c - low.astype(F32)).astype(F32)
    return low, high, frac, valid


def _scale_tables(pixel, batch_index, stride, H, W, base_px):
    """Per-job tables for one scale. base_px = pixel offset of this scale's
    image block in the channels-last concatenated feature tensor."""
    A = pixel.shape[0]
    st = F32(stride)
    half = F32(SIZE / 2.0)
    centers = (np.arange(SIZE, dtype=F32) + F32(0.5)).astype(F32)

    px = pixel[:, 0].astype(F32)
    py = pixel[:, 1].astype(F32)
    x1 = _fake_quant(np.maximum(px / st - half, F32(0.0)).astype(F32))
    y1 = _fake_quant(np.maximum(py / st - half, F32(0.0)).astype(F32))
    x2 = _fake_quant(np.maximum(px / st + half, F32(0.0)).astype(F32))
    y2 = _fake_quant(np.maximum(py / st + half, F32(0.0)).astype(F32))
    roi_w = np.maximum(x2 - x1, F32(1.0)).astype(F32)
    roi_h = np.maximum(y2 - y1, F32(1.0)).astype(F32)
    y = (y1[:, None] + centers[None, :] * (roi_h / F32(SIZE))[:, None]).astype(F32)
    x = (x1[:, None] + centers[None, :] * (roi_w / F32(SIZE))[:, None]).astype(F32)

    yl, yh, fy, vy = _prep(y, H)
    xl, xh, fx, vx = _prep(x, W)

    b = batch_index.astype(np.int64)

    # y window + Wy (y-interp matrix with validity folded in)
    wy0 = np.minimum(yl[:, 0], H - ROWS).astype(np.int64)
    ey_lo = yl.astype(np.int64) - wy0[:, None]
    ey_hi = yh.astype(np.int64) - wy0[:, None]
    assert ey_lo.min() >= 0 and ey_lo.max() <= 8
    assert ey_hi.min() >= 0 and ey_hi.max() <= 8
    vyf = vy.astype(F32)
    wl = ((F32(1.0) - fy) * vyf).astype(F32)
    wh = (fy * vyf).astype(F32)
    Wy = np.zeros((A, ROWS, SIZE), F32)
    aa = np.repeat(np.arange(A), SIZE)
    ii = np.tile(np.arange(SIZE), A)
    np.add.at(Wy, (aa, ey_lo.ravel(), ii), wl.ravel())
    np.add.at(Wy, (aa, ey_hi.ravel(), ii), wh.ravel())

    # x window
    ar = np.arange(SIZE, dtype=np.int32)
    reg = (np.all(xl == xl[:, :1] + ar[None, :], axis=1)
           & np.all(xh == xl + 1, axis=1)
           & np.all(vx, axis=1)
           & np.all(fx == fx[:, :1], axis=1))
    xmin = np.minimum(xl.min(axis=1), xh.min(axis=1)).astype(np.int64)
    x0w = np.clip(xmin & ~1, 0, W - PXW)
    q0 = (xmin - x0w).astype(np.int64)          # parity offset for reg jobs
    ex_lo = xl.astype(np.int64) - x0w[:, None]
    ex_hi = xh.astype(np.int64) - x0w[:, None]
    assert ex_lo.min() >= 0 and ex_lo.max() <= PXW - 1
    assert ex_hi.min() >= 0 and ex_hi.max() <= PXW - 1
    assert np.all(q0[reg] <= 1)
    bx0 = (F32(1.0) - fx[:, 0]).astype(F32)
    bx1 = fx[:, 0].astype(F32)

    # irr: dense x matrix over the 10-px window (validity folded in)
    vxf = vx.astype(F32)
    Mx = np.zeros((A, PXW, SIZE), F32)
    jj = np.tile(np.arange(SIZE), A)
    np.add.at(Mx, (aa, ex_lo.ravel(), jj), ((F32(1.0) - fx) * vxf).ravel())
    np.add.at(Mx, (aa, ex_hi.ravel(), jj), (fx * vxf).ravel())

    # window start unit per row e: u0 + e*(W//2)
    u0 = (base_px + (b * H + wy0) * W + x0w) // 2
    return dict(Wy=Wy, reg=reg, q0=q0, bx0=bx0, bx1=bx1, Mx=Mx,
                u0=u0.astype(np.int64), wrow=W // 2)


def _host_prep(f0, f1, f2, pixel, batch_index):
    A = pixel.shape[0]
    feats = (f0, f1, f2)

    cat = np.concatenate([
        np.ascontiguousarray(np.asarray(f, dtype=F32).transpose(0, 2, 3, 1))
        .reshape(-1, C) for f in feats], axis=0).astype(np.float16)
    nunits = cat.shape[0] // 2
    nreg = (nunits - 1) // REGION + 1
    nunits_pad = (nreg - 1) * REGION + 32768 + 8
    cat2 = np.zeros((nunits_pad, UNIT), np.float16)
    cat2[:nunits] = cat.reshape(nunits, UNIT)

    tabs = []
    base_px = 0
    for s, f in enumerate(feats):
        H, W = f.shape[2], f.shape[3]
        tabs.append(_scale_tables(np.asarray(pixel, F32),
                                  np.asarray(batch_index), STRIDES[s],
                                  H, W, base_px))
        base_px += 4 * H * W

    # ---- flat job lists with stratum key (region, kind)
    jobs = []            # (s, a, region, parity, is_reg)
    for s in range(3):
        t = tabs[s]
        regions = t["u0"] // REGION
        for a in range(A):
            jobs.append((s, a, int(regions[a]), int(t["q0"][a]),
                         bool(t["reg"][a])))

    # stratified round-robin assignment:
    #   reg strata key (region, parity), irr strata key (region)
    core_reg = {}        # (region, parity) -> [list per core of (s,a)]
    core_irr = {}        # region -> [list per core]
    cnt_reg = {}
    cnt_irr = {}
    for (s, a, r, q, isreg) in jobs:
        if isreg:
            key = (r, q)
            lst = core_reg.setdefault(key, [[] for _ in range(N_CORES)])
            k = cnt_reg[key] = cnt_reg.get(key, 0) + 1
            lst[(k - 1) % N_CORES].append((s, a))
        else:
            lst = core_irr.setdefault(r, [[] for _ in range(N_CORES)])
            k = cnt_irr[r] = cnt_irr.get(r, 0) + 1
            lst[(k - 1) % N_CORES].append((s, a))

    # global group counts per stratum (max over cores)
    reg_strata = sorted(core_reg.keys())
    irr_strata = sorted(core_irr.keys())
    NB = {key: max(-(-len(lst) // JOBS_PG) for lst in core_reg[key])
          for key in reg_strata}
    NP_ = {r: max(-(-len(lst) // 2) for lst in core_irr[r])
           for r in irr_strata}
    NRGtot = sum(NB.values())
    NPtot = sum(NP_.values())

    # batches (static program structure)
    reg_batches = []     # (region, parity, n_groups_in_batch, gi0)
    gi = 0
    for (r, q) in reg_strata:
        nb = NB[(r, q)]
        while nb > 0:
            g = min(nb, NGMAX)
            reg_batches.append((r, q, g, gi))
            gi += g
            nb -= g
    irr_batches = []     # (region, n_slots_in_batch, pi0)
    pi = 0
    for r in irr_strata:
        npr = NP_[r]
        while npr > 0:
            sct = min(npr, NSMAX)
            irr_batches.append((r, sct, pi))
            pi += sct
            npr -= sct

    # ---- per-core packed device inputs + output mapping
    RCOLS = sum(8 * g for (_, _, g, _) in reg_batches)
    ICOLS = sum(8 * sct for (_, sct, _) in irr_batches)
    per_core = []
    for ccc in range(N_CORES):
        ridx = np.zeros((128, max(RCOLS, 1)), np.int16)
        rw = np.zeros((128, max(NRGtot, 1) * 112), np.float16)
        rs = np.zeros((128, max(NRGtot, 1) * 2), F32)
        iidx = np.zeros((128, max(ICOLS, 1)), np.int16)
        iw = np.zeros((128, max(NPtot, 1) * 256), np.float16)
        rmap = []        # (gi, k, s, a)
        imap = []        # (pi, k2, s, a)

        col0 = 0
        for (r, q, G, gi0) in reg_batches:
            lst = core_reg[(r, q)][ccc]
            idxvals = np.zeros((G * 128,), np.int64)
            sg0 = _stratum_gi0(reg_batches, r, q)
            for g in range(G):
                for k in range(JOBS_PG):
                    j = (gi0 - sg0 + g) * JOBS_PG + k
                    if j < len(lst):
                        s, a = lst[j]
                        t = tabs[s]
                        gidx = gi0 + g
                        rmap.append((gidx, k, s, a))
                        rel0 = int(t["u0"][a] - r * REGION)
                        for e in range(ROWS):
                            idxvals[g * 128 + k * ROWS + e] = rel0 + e * t["wrow"]
                        rw[k * ROWS:(k + 1) * ROWS,
                           gidx * 112 + k * SIZE: gidx * 112 + (k + 1) * SIZE] = \
                            t["Wy"][a].astype(np.float16)
                        rs[k * ROWS:(k + 1) * ROWS, gidx * 2] = t["bx0"][a]
                        rs[k * ROWS:(k + 1) * ROWS, gidx * 2 + 1] = t["bx1"][a]
            assert idxvals.max() < 32768
            cols = G * 8
            tilecols = _wrap_idx(idxvals)
            ridx[:, col0: col0 + cols] = tilecols
            col0 += cols

        col0 = 0
        for (r, S, pi0) in irr_batches:
            lst = core_irr[r][ccc]
            idxvals = np.zeros((S * 128,), np.int64)
            for sl in range(S):
                for k2 in range(2):
                    j = (pi0 - _irr_pi0(irr_batches, r) + sl) * 2 + k2
                    if j < len(lst):
                        s, a = lst[j]
                        t = tabs[s]
                        pidx = pi0 + sl
                        imap.append((pidx, k2, s, a))
                        rel0 = int(t["u0"][a] - r * REGION)
                        for e in range(ROWS):
                            for u in range(PXW // 2):
                                idxvals[sl * 128 + k2 * 45 + e * 5 + u] = \
                                    rel0 + e * t["wrow"] + u
                        # W2 par matrices [45, 64] each
                        Wy = t["Wy"][a]                     # [9, 8]
                        Mx = t["Mx"][a]                     # [10, 8]
                        for par in range(2):
                            Mp = Mx[par::2]                 # [5, 8]
                            W2 = np.einsum("ei,uj->euij", Wy, Mp).reshape(45, 64)
                            iw[k2 * 45:(k2 + 1) * 45,
                               pidx * 256 + par * 128 + k2 * 64:
                               pidx * 256 + par * 128 + (k2 + 1) * 64] = \
                                W2.astype(np.float16)
            assert idxvals.max() < 32768
            cols = S * 8
            iidx[:, col0: col0 + cols] = _wrap_idx(idxvals)
            col0 += cols

        per_core.append(dict(ridx=ridx, rw=rw, rs=rs, iidx=iidx, iw=iw,
                             rmap=rmap, imap=imap))

    return dict(cat2=cat2, per_core=per_core, tabs=tabs,
                reg_batches=reg_batches, irr_batches=irr_batches,
                NRGtot=NRGtot, NPtot=NPtot, RCOLS=RCOLS, ICOLS=ICOLS,
                nunits_pad=nunits_pad, A=A)


def _stratum_gi0(reg_batches, r, q):
    for (rr, qq, G, gi0) in reg_batches:
        if (rr, qq) == (r, q):
            return gi0
    raise KeyError


def _irr_pi0(irr_batches, r):
    for (rr, S, pi0) in irr_batches:
        if rr == r:
            return pi0
    raise KeyError


def _wrap_idx(idxvals):
    """[N] linear idx values -> [128, N//16] int16 tile (16-partition wrap,
    replicated to all 128 partitions)."""
    n = len(idxvals)
    assert n % 16 == 0
    tile = np.zeros((128, n // 16), np.int16)
    lin = np.asarray(idxvals, np.int64)
    assert lin.min() >= 0 and lin.max() < 32768
    t16 = lin.reshape(n // 16, 16).T.astype(np.int16)   # [16, n/16]
    for rep in range(8):
        tile[rep * 16:(rep + 1) * 16, :] = t16
    return tile


# ----------------------------------------------------------------------------
# device program
# ----------------------------------------------------------------------------

def _build_program(prep):
    f16 = mybir.dt.float16
    f32 = mybir.dt.float32
    i16 = mybir.dt.int16
    COPY = mybir.ActivationFunctionType.Copy
    MULT = mybir.AluOpType.mult
    ADD = mybir.AluOpType.add
    NRGtot = prep["NRGtot"]
    NPtot = prep["NPtot"]
    RCOLS = prep["RCOLS"]
    ICOLS = prep["ICOLS"]
    NUP = prep["nunits_pad"]

    nc = bacc.Bacc("TRN2", num_swdge_queues=4)
    cat2_t = nc.dram_tensor("cat2", [NUP, UNIT], f16, kind="ExternalInput")
    ridx_t = nc.dram_tensor("ridx", [128, max(RCOLS, 1)], i16,
                            kind="ExternalInput")
    rw_t = nc.dram_tensor("rw", [128, max(NRGtot, 1) * 112], f16,
                          kind="ExternalInput")
    rs_t = nc.dram_tensor("rs", [128, max(NRGtot, 1) * 2], f32,
                          kind="ExternalInput")
    iidx_t = nc.dram_tensor("iidx", [128, max(ICOLS, 1)], i16,
                            kind="ExternalInput")
    iw_t = nc.dram_tensor("iw", [128, max(NPtot, 1) * 256], f16,
                          kind="ExternalInput")
    oreg_t = nc.dram_tensor("out_reg", [128, max(NRGtot, 1) * 512], f16,
                            kind="ExternalOutput")
    oirr_t = nc.dram_tensor("out_irr", [128, max(NPtot, 1) * 64], f16,
                            kind="ExternalOutput")

    with TileContext(nc) as tc:
        with tc.tile_pool(name="const", bufs=1) as cpool, \
             tc.tile_pool(name="gat", bufs=3) as gpool, \
             tc.tile_pool(name="mv", bufs=8) as vpool, \
             tc.tile_pool(name="ps", bufs=4, space="PSUM") as pspool, \
             tc.tile_pool(name="ob", bufs=3) as obpool:

            ridx = cpool.tile([128, max(RCOLS, 1)], i16)
            rw = cpool.tile([128, max(NRGtot, 1) * 112], f16)
            rs = cpool.tile([128, max(NRGtot, 1) * 2], f32)
            iidx = cpool.tile([128, max(ICOLS, 1)], i16)
            iw = cpool.tile([128, max(NPtot, 1) * 256], f16)
            nc.sync.dma_start(ridx[:, :], ridx_t[:, :])
            nc.sync.dma_start(rw[:, :], rw_t[:, :])
            nc.sync.dma_start(rs[:, :], rs_t[:, :])
            nc.sync.dma_start(iidx[:, :], iidx_t[:, :])
            nc.sync.dma_start(iw[:, :], iw_t[:, :])

            def region_ap(r, esize):
                return bass.AP(cat2_t, r * REGION * UNIT,
                               [(UNIT, 32768), (1, esize)])

            qrr = 0
            col0 = 0
            for (r, q, G, gi0) in prep["reg_batches"]:
                Gt = gpool.tile([128, G * ESIZE], f16, tag="gt")
                nc.gpsimd.dma_gather(
                    out_ap=Gt[:, :].rearrange("p (s e) -> p s e", e=ESIZE),
                    in_ap=region_ap(r, ESIZE),
                    idxs_ap=ridx[:, col0: col0 + G * 8],
                    num_idxs=G * 128, num_idxs_reg=G * 128,
                    elem_size=ESIZE, elem_step=UNIT,
                    queue_num=qrr % 4)
                qrr += 1
                col0 += G * 8
                obuf = obpool.tile([128, G * 512], f16, tag="ob")
                for g in range(G):
                    gi = gi0 + g
                    base = g * ESIZE + 64 * q
                    m = vpool.tile([128, 512], f16, tag="m")
                    nc.scalar.activation(m[:, :], Gt[:, base: base + 512],
                                         COPY, scale=rs[:, 2 * gi: 2 * gi + 1])
                    v = vpool.tile([128, 512], f16, tag="v")
                    nc.vector.scalar_tensor_tensor(
                        out=v[:, :], in0=Gt[:, base + 64: base + 576],
                        scalar=rs[:, 2 * gi + 1: 2 * gi + 2], in1=m[:, :],
                        op0=MULT, op1=ADD)
                    u = pspool.tile([112, 512], f32, tag="u")
                    nc.tensor.matmul(out=u[:, :],
                                     lhsT=rw[:, gi * 112: (gi + 1) * 112],
                                     rhs=v[:, :], start=True, stop=True)
                    nc.scalar.activation(obuf[:112, g * 512: (g + 1) * 512],
                                         u[:, :], COPY)
                nc.sync.dma_start(oreg_t[:, gi0 * 512: (gi0 + G) * 512],
                                  obuf[:, :])

            col0 = 0
            for (r, S, pi0) in prep["irr_batches"]:
                Git = gpool.tile([128, S * UNIT], f16, tag="git")
                nc.gpsimd.dma_gather(
                    out_ap=Git[:, :].rearrange("p (s e) -> p s e", e=UNIT),
                    in_ap=region_ap(r, UNIT),
                    idxs_ap=iidx[:, col0: col0 + S * 8],
                    num_idxs=S * 128, num_idxs_reg=S * 128,
                    elem_size=UNIT, elem_step=UNIT,
                    queue_num=qrr % 4)
                qrr += 1
                col0 += S * 8
                iob = obpool.tile([128, S * 64], f16, tag="iob")
                for sl in range(S):
                    pi = pi0 + sl
                    u2 = pspool.tile([128, 64], f32, tag="u2")
                    nc.tensor.matmul(
                        out=u2[:, :],
                        lhsT=iw[:, pi * 256: pi * 256 + 128],
                        rhs=Git[:, sl * UNIT: sl * UNIT + 64],
                        start=True, stop=False)
                    nc.tensor.matmul(
                        out=u2[:, :],
                        lhsT=iw[:, pi * 256 + 128: pi * 256 + 256],
                        rhs=Git[:, sl * UNIT + 64: sl * UNIT + 128],
                        start=False, stop=True)
                    nc.vector.tensor_copy(out=iob[:, sl * 64: (sl + 1) * 64],
                                          in_=u2[:, :])
                nc.sync.dma_start(oirr_t[:, pi0 * 64: (pi0 + S) * 64],
                                  iob[:, :])

    nc.finalize()
    return nc


# ----------------------------------------------------------------------------
# entry point
# ----------------------------------------------------------------------------

def kernel(f0, f1, f2, pixel, batch_index):
    global LAST_RESULTS
    prep = _host_prep(f0, f1, f2, pixel, batch_index)
    A = prep["A"]

    nc = _build_program(prep)

    in_maps = []
    for ccc in range(N_CORES):
        pc = prep["per_core"][ccc]
        in_maps.append({"cat2": prep["cat2"], "ridx": pc["ridx"],
                        "rw": pc["rw"], "rs": pc["rs"],
                        "iidx": pc["iidx"], "iw": pc["iw"]})

    res = run_bass_kernel_spmd(nc, in_maps, core_ids=list(range(N_CORES)),
                               trace=bool(os.environ.get("BASS_TRACE")))
    LAST_RESULTS = res

    out = np.zeros((A, 3, C, SIZE, SIZE), F32)
    NRGtot, NPtot = prep["NRGtot"], prep["NPtot"]
    for ccc in range(N_CORES):
        pc = prep["per_core"][ccc]
        raw = res.results[ccc]["out_reg"].astype(F32)
        # [128, NRG*512] -> [14jobs, 8i, NRG, 8j, 64c] -> [14, NRG, 64, 8, 8]
        rr = (raw[:112].reshape(JOBS_PG, SIZE, NRGtot, SIZE, C)
              .transpose(0, 2, 4, 1, 3))
        if pc["rmap"]:
            gia = np.array([m[0] for m in pc["rmap"]])
            ka = np.array([m[1] for m in pc["rmap"]])
            sa = np.array([m[2] for m in pc["rmap"]])
            aa = np.array([m[3] for m in pc["rmap"]])
            out[aa, sa] = rr[ka, gia]
        if pc["imap"]:
            rawi = res.results[ccc]["out_irr"].astype(F32)
            # [128, NP*64]: part = k2*64 + i*8 + j -> [2, 8i, 8j, NP, 64c]
            ri = (rawi.reshape(2, SIZE, SIZE, NPtot, C)
                  .transpose(0, 3, 4, 1, 2))
            pia = np.array([m[0] for m in pc["imap"]])
            k2a = np.array([m[1] for m in pc["imap"]])
            sa = np.array([m[2] for m in pc["imap"]])
            aa = np.array([m[3] for m in pc["imap"]])
            out[aa, sa] = ri[k2a, pia]
    return out.reshape(A, 3 * C, SIZE, SIZE)


# ----------------------------------------------------------------------------
# numpy emulation of the device program (for offline validation)
# ----------------------------------------------------------------------------

def emulate(f0, f1, f2, pixel, batch_index):
    prep = _host_prep(f0, f1, f2, pixel, batch_index)
    A = prep["A"]
    cat2 = prep["cat2"]
    flat = cat2.reshape(-1)
    NRGtot, NPtot = prep["NRGtot"], prep["NPtot"]
    out = np.zeros((A, 3, C, SIZE, SIZE), F32)
    for ccc in range(N_CORES):
        pc = prep["per_core"][ccc]
        raw = np.zeros((128, max(NRGtot, 1) * 512), np.float16)
        col0 = 0
        for (r, q, G, gi0) in prep["reg_batches"]:
            # gather
            Gt = np.zeros((128, G * ESIZE), np.float16)
            for i in range(G * 128):
                p, sslot = i % 128, i // 128
                idx = int(pc["ridx"][i % 16, col0 + i // 16])
                st = (r * REGION + idx) * UNIT
                Gt[p, sslot * ESIZE: (sslot + 1) * ESIZE] = flat[st: st + ESIZE]
            col0 += G * 8
            for g in range(G):
                gi = gi0 + g
                base = g * ESIZE + 64 * q
                g32 = Gt.astype(F32)
                m = (g32[:, base: base + 512]
                     * pc["rs"][:, 2 * gi: 2 * gi + 1]).astype(np.float16)
                v = (g32[:, base + 64: base + 576]
                     * pc["rs"][:, 2 * gi + 1: 2 * gi + 2]
                     + m.astype(F32)).astype(np.float16)
                u = (pc["rw"][:, gi * 112: (gi + 1) * 112].astype(F32).T
                     @ v.astype(F32))
                raw[:112, gi * 512: (gi + 1) * 512] = u.astype(np.float16)
        rr = (raw[:112].astype(F32).reshape(JOBS_PG, SIZE, NRGtot, SIZE, C)
              .transpose(0, 2, 4, 1, 3))
        if pc["rmap"]:
            gia = np.array([m[0] for m in pc["rmap"]])
            ka = np.array([m[1] for m in pc["rmap"]])
            sa = np.array([m[2] for m in pc["rmap"]])
            aa = np.array([m[3] for m in pc["rmap"]])
            out[aa, sa] = rr[ka, gia]

        rawi = np.zeros((128, max(NPtot, 1) * 64), np.float16)
        col0 = 0
        for (r, S, pi0) in prep["irr_batches"]:
            Git = np.zeros((128, S * UNIT), np.float16)
            for i in range(S * 128):
                p, sslot = i % 128, i // 128
                idx = int(pc["iidx"][i % 16, col0 + i // 16])
                st = (r * REGION + idx) * UNIT
                Git[p, sslot * UNIT: (sslot + 1) * UNIT] = flat[st: st + UNIT]
            col0 += S * 8
            for sl in range(S):
                pi = pi0 + sl
                u2 = (pc["iw"][:, pi * 256: pi * 256 + 128].astype(F32).T
                      @ Git[:, sl * UNIT: sl * UNIT + 64].astype(F32))
                u2 += (pc["iw"][:, pi * 256 + 128: pi * 256 + 256]
                       .astype(F32).T
                       @ Git[:, sl * UNIT + 64: sl * UNIT + 128].astype(F32))
                rawi[:, pi * 64: (pi + 1) * 64] = u2.astype(np.float16)
        if pc["imap"]:
            ri = (rawi.astype(F32).reshape(2, SIZE, SIZE, NPtot, C)
                  .transpose(0, 3, 4, 1, 2))
            pia = np.array([m[0] for m in pc["imap"]])
            k2a = np.array([m[1] for m in pc["imap"]])
            sa = np.array([m[2] for m in pc["imap"]])
            aa = np.array([m[3] for m in pc["imap"]])
            out[aa, sa] = ri[k2a, pia]
    return out.reshape(A, 3 * C, SIZE, SIZE)



# revision 6
# speedup vs baseline: 1.5078x; 1.3856x over previous
"""Trainium2 Bass kernel for nn_CropperQAT (multi-scale RoIAlign with
fake-quantized rois) — v3.

Strategy (data-parallel over (roi, scale) jobs, 8 cores):
  * Host replicates the reference roi math bit-exactly (numpy f32), then for
    each job (roi a, scale s) derives a 9-row x 10-px fp16 feature window
    (all bilinear taps fit) plus interpolation weights.
  * Feature rows are RIGHT-PADDED by 8 replicated edge pixels so rois whose
    x taps clamp at the right edge stay on the regular (unit-stride) path;
    invalid (x > W) bins are zeroed during host unpack. Only left-clamped
    rois (x1 == 0 with roi_w < 8, non-unit stride) take the irregular path.
  * Features are stored channels-last fp16 as 2-px "units" (256B) so
    dma_gather (int16 indices, 256B-stride) can fetch windows; the unit
    space is split into regions so relative indices fit int16. Gathers are
    spread round-robin over 4 SWDGE queues so descriptor generation
    parallelizes across Q7 core pairs.
  * Device, x-regular jobs (partition = (job k, window row e), 14x9=126):
      - dma_gather batches up to 8 groups: [128, G*640] fp16
      - x-interp: ONE DVE scalar_tensor_tensor per group:
        t = G0 + r*G1 with r = fx/(1-fx); the (1-fx) factor is folded into
        the y-interp matrix Wy host-side.
      - y-interp: PE matmul with per-group block-diagonal [126->112]
        fp16 weights (handles y clamping/validity for free) -> PSUM f32
      - ACT copy PSUM -> fp16 output tile; batched DMA out.
  * x-irregular jobs (~4%): full bilinear as two accumulating matmuls
    (even/odd pixel parity) per 2-job pair slot; zero DVE work.
  * Host converts fp16 device output to f32 (fp16 path measures ~5e-4
    rel err; tolerance 2e-2).
"""
import os
import sys

sys.path.insert(0, "/opt/trn_rl_repo")

import numpy as np

import concourse.bass as bass
import concourse.bacc as bacc
import concourse.mybir as mybir
from concourse.tile import TileContext
from concourse.bass_utils import run_bass_kernel_spmd

F32 = np.float32
SIZE = 8
STRIDES = (4, 8, 16)
QS = np.float32(0.25)
C = 64
N_CORES = 8
JOBS_PG = 14                 # jobs per regular group (x9 rows = 126 partitions)
ROWS = 9                     # y window rows
PXW = 10                     # x window pixels (5 units)
XPAD = 8                     # replicated right-pad pixels per row
UNIT = 128                   # fp16 elems per unit (2 px * 64 ch)
ESIZE = PXW * C              # 640 fp16 per gathered row-window element
REGION = 30400               # units per region (int16 headroom: +1061 < 32768)
NGMAX = 8                    # groups per dma_gather batch
NSMAX = 8                    # irr pair-slots per dma_gather batch
NQ = 4                       # SWDGE queues for gather desc-gen parallelism

LAST_RESULTS = None


# ----------------------------------------------------------------------------
# host-side math (bit-exact replication of the jax reference)
# ----------------------------------------------------------------------------

def _fake_quant(x):
    return (np.clip(np.round(x / QS), -32768, 32767) * QS).astype(F32)


def _prep(c, L):
    valid = (c >= -1.0) & (c <= L)
    c = np.maximum(c, F32(0.0))
    low0 = np.floor(c).astype(np.int32)
    hi_edge = low0 >= L - 1
    low = np.where(hi_edge, L - 1, low0).astype(np.int32)
    high = np.where(hi_edge, L - 1, low0 + 1).astype(np.int32)
    c = np.where(hi_edge, F32(L - 1), c).astype(F32)
    frac = (c - low.astype(F32)).astype(F32)
    return low, high, frac, valid


def _scale_tables(pixel, batch_index, stride, H, W, base_px):
    """Per-job tables for one scale. base_px = pixel offset of this scale's
    image block in the channels-last concatenated (row-padded) tensor."""
    A = pixel.shape[0]
    Wp = W + XPAD
    st = F32(stride)
    half = F32(SIZE / 2.0)
    centers = (np.arange(SIZE, dtype=F32) + F32(0.5)).astype(F32)

    px = pixel[:, 0].astype(F32)
    py = pixel[:, 1].astype(F32)
    x1 = _fake_quant(np.maximum(px / st - half, F32(0.0)).astype(F32))
    y1 = _fake_quant(np.maximum(py / st - half, F32(0.0)).astype(F32))
    x2 = _fake_quant(np.maximum(px / st + half, F32(0.0)).astype(F32))
    y2 = _fake_quant(np.maximum(py / st + half, F32(0.0)).astype(F32))
    roi_w = np.maximum(x2 - x1, F32(1.0)).astype(F32)
    roi_h = np.maximum(y2 - y1, F32(1.0)).astype(F32)
    y = (y1[:, None] + centers[None, :] * (roi_h / F32(SIZE))[:, None]).astype(F32)
    x = (x1[:, None] + centers[None, :] * (roi_w / F32(SIZE))[:, None]).astype(F32)

    yl, yh, fy, vy = _prep(y, H)
    xl, xh, fx, vx = _prep(x, W)

    b = batch_index.astype(np.int64)

    # y window + Wy (y-interp matrix with validity folded in)
    wy0 = np.minimum(yl[:, 0], H - ROWS).astype(np.int64)
    ey_lo = yl.astype(np.int64) - wy0[:, None]
    ey_hi = yh.astype(np.int64) - wy0[:, None]
    assert ey_lo.min() >= 0 and ey_lo.max() <= 8
    assert ey_hi.min() >= 0 and ey_hi.max() <= 8
    vyf = vy.astype(F32)
    wl = ((F32(1.0) - fy) * vyf).astype(F32)
    wh = (fy * vyf).astype(F32)
    Wy = np.zeros((A, ROWS, SIZE), F32)
    aa = np.repeat(np.arange(A), SIZE)
    ii = np.tile(np.arange(SIZE), A)
    np.add.at(Wy, (aa, ey_lo.ravel(), ii), wl.ravel())
    np.add.at(Wy, (aa, ey_hi.ravel(), ii), wh.ravel())

    # --- regular-x classification against the PADDED row layout --------
    # virtual unit-stride taps a_j = xv0+j, b_j = a_j+1 into the padded row
    # (positions >= W-1 all hold F[W-1]); fx0 constant.
    x0f = (x1 + F32(0.5)).astype(F32)
    xv0 = np.floor(x0f).astype(np.int64)
    fx0 = (x0f - xv0.astype(F32)).astype(F32)
    jj8 = np.arange(SIZE, dtype=np.int64)
    a_j = xv0[:, None] + jj8[None, :]
    b_j = a_j + 1
    a_eff = np.minimum(a_j, W - 1)
    b_eff = np.minimum(b_j, W - 1)
    interior = (a_eff == xl) & (b_eff == xh) & (fx == fx0[:, None])
    hi_coll = (xl == W - 1) & (xh == W - 1) & (a_j >= W - 1)
    match = interior | hi_coll | (~vx)
    reg = ((roi_w == F32(8.0)) & np.all(match, axis=1)
           & (xv0 >= 0) & (xv0 + SIZE + 1 <= Wp - 1))
    # fold (1-fx0) into Wy for reg jobs; r = fx0/(1-fx0) as the stt scalar
    bx0 = (F32(1.0) - fx0).astype(F32)
    rsc = (fx0 / bx0).astype(F32)
    x0w = np.where(reg, xv0 & ~np.int64(1), 0)
    q0 = (xv0 - x0w).astype(np.int64)
    assert np.all(q0[reg] >= 0) and np.all(q0[reg] <= 1)
    assert np.all(x0w[reg] + PXW <= Wp)

    # host-side x-validity mask (reg jobs only; irr folds vx into Mx)
    xmask = (~vx) & reg[:, None]

    # irr: dense x matrix over the 10-px window at x0w=0 (validity folded)
    vxf = vx.astype(F32)
    Mx = np.zeros((A, PXW, SIZE), F32)
    irr_a = np.nonzero(~reg)[0]
    if irr_a.size:
        ex_lo = xl[irr_a].astype(np.int64)
        ex_hi = xh[irr_a].astype(np.int64)
        assert ex_lo.min() >= 0 and ex_lo.max() <= PXW - 1
        assert ex_hi.min() >= 0 and ex_hi.max() <= PXW - 1
        ai = np.repeat(irr_a, SIZE)
        ji = np.tile(np.arange(SIZE), irr_a.size)
        np.add.at(Mx, (ai, ex_lo.ravel(), ji),
                  ((F32(1.0) - fx[irr_a]) * vxf[irr_a]).ravel())
        np.add.at(Mx, (ai, ex_hi.ravel(), ji),
                  (fx[irr_a] * vxf[irr_a]).ravel())

    # window start unit per row e: u0 + e*(Wp//2)
    u0 = (base_px + (b * H + wy0) * Wp + x0w) // 2
    return dict(Wy=Wy, reg=reg, q0=q0, bx0=bx0, rsc=rsc, Mx=Mx,
                u0=u0.astype(np.int64), wrow=Wp // 2, xmask=xmask)


def _host_prep(f0, f1, f2, pixel, batch_index):
    A = pixel.shape[0]
    feats = (f0, f1, f2)

    # channels-last rows, right-padded with XPAD copies of the edge pixel
    blocks = []
    for f in feats:
        t = np.asarray(f, dtype=F32).transpose(0, 2, 3, 1)    # [N,H,W,C]
        pad = np.repeat(t[:, :, -1:, :], XPAD, axis=2)
        blocks.append(np.concatenate([t, pad], axis=2).reshape(-1, C))
    cat = np.concatenate(blocks, axis=0).astype(np.float16)
    nunits = cat.shape[0] // 2
    nreg = (nunits - 1) // REGION + 1
    nunits_pad = (nreg - 1) * REGION + 32768 + 8
    cat2 = np.zeros((nunits_pad, UNIT), np.float16)
    cat2[:nunits] = cat.reshape(nunits, UNIT)

    tabs = []
    base_px = 0
    for s, f in enumerate(feats):
        H, W = f.shape[2], f.shape[3]
        tabs.append(_scale_tables(np.asarray(pixel, F32),
                                  np.asarray(batch_index), STRIDES[s],
                                  H, W, base_px))
        base_px += 4 * H * (W + XPAD)

    # ---- flat job lists with stratum key (region, kind)
    jobs = []            # (s, a, region, parity, is_reg)
    for s in range(3):
        t = tabs[s]
        regions = t["u0"] // REGION
        for a in range(A):
            jobs.append((s, a, int(regions[a]), int(t["q0"][a]),
                         bool(t["reg"][a])))

    # stratified round-robin assignment:
    #   reg strata key (region, parity), irr strata key (region)
    core_reg = {}        # (region, parity) -> [list per core of (s,a)]
    core_irr = {}        # region -> [list per core]
    cnt_reg = {}
    cnt_irr = {}
    for (s, a, r, q, isreg) in jobs:
        if isreg:
            key = (r, q)
            lst = core_reg.setdefault(key, [[] for _ in range(N_CORES)])
            k = cnt_reg[key] = cnt_reg.get(key, 0) + 1
            lst[(k - 1) % N_CORES].append((s, a))
        else:
            lst = core_irr.setdefault(r, [[] for _ in range(N_CORES)])
            k = cnt_irr[r] = cnt_irr.get(r, 0) + 1
            lst[(k - 1) % N_CORES].append((s, a))

    # global group counts per stratum (max over cores)
    reg_strata = sorted(core_reg.keys())
    irr_strata = sorted(core_irr.keys())
    NB = {key: max(-(-len(lst) // JOBS_PG) for lst in core_reg[key])
          for key in reg_strata}
    NP_ = {r: max(-(-len(lst) // 2) for lst in core_irr[r])
           for r in irr_strata}
    NRGtot = sum(NB.values())
    NPtot = sum(NP_.values())

    # batches (static program structure)
    reg_batches = []     # (region, parity, n_groups_in_batch, gi0)
    gi = 0
    for (r, q) in reg_strata:
        nb = NB[(r, q)]
        while nb > 0:
            g = min(nb, NGMAX)
            reg_batches.append((r, q, g, gi))
            gi += g
            nb -= g
    irr_batches = []     # (region, n_slots_in_batch, pi0)
    pi = 0
    for r in irr_strata:
        npr = NP_[r]
        while npr > 0:
            sct = min(npr, NSMAX)
            irr_batches.append((r, sct, pi))
            pi += sct
            npr -= sct

    # ---- per-core packed device inputs + output mapping
    RCOLS = sum(8 * g for (_, _, g, _) in reg_batches)
    ICOLS = sum(8 * sct for (_, sct, _) in irr_batches)
    per_core = []
    for ccc in range(N_CORES):
        ridx = np.zeros((128, max(RCOLS, 1)), np.int16)
        rw = np.zeros((128, max(NRGtot, 1) * 112), np.float16)
        rs = np.zeros((128, max(NRGtot, 1)), F32)
        iidx = np.zeros((128, max(ICOLS, 1)), np.int16)
        iw = np.zeros((128, max(NPtot, 1) * 256), np.float16)
        rmap = []        # (gi, k, s, a)
        imap = []        # (pi, k2, s, a)

        col0 = 0
        for (r, q, G, gi0) in reg_batches:
            lst = core_reg[(r, q)][ccc]
            idxvals = np.zeros((G * 128,), np.int64)
            sg0 = _stratum_gi0(reg_batches, r, q)
            for g in range(G):
                for k in range(JOBS_PG):
                    j = (gi0 - sg0 + g) * JOBS_PG + k
                    if j < len(lst):
                        s, a = lst[j]
                        t = tabs[s]
                        gidx = gi0 + g
                        rmap.append((gidx, k, s, a))
                        rel0 = int(t["u0"][a] - r * REGION)
                        for e in range(ROWS):
                            idxvals[g * 128 + k * ROWS + e] = rel0 + e * t["wrow"]
                        rw[k * ROWS:(k + 1) * ROWS,
                           gidx * 112 + k * SIZE: gidx * 112 + (k + 1) * SIZE] = \
                            (t["Wy"][a] * t["bx0"][a]).astype(np.float16)
                        rs[k * ROWS:(k + 1) * ROWS, gidx] = t["rsc"][a]
            assert idxvals.max() < 32768
            cols = G * 8
            tilecols = _wrap_idx(idxvals)
            ridx[:, col0: col0 + cols] = tilecols
            col0 += cols

        col0 = 0
        for (r, S, pi0) in irr_batches:
            lst = core_irr[r][ccc]
            idxvals = np.zeros((S * 128,), np.int64)
            for sl in range(S):
                for k2 in range(2):
                    j = (pi0 - _irr_pi0(irr_batches, r) + sl) * 2 + k2
                    if j < len(lst):
                        s, a = lst[j]
                        t = tabs[s]
                        pidx = pi0 + sl
                        imap.append((pidx, k2, s, a))
                        rel0 = int(t["u0"][a] - r * REGION)
                        for e in range(ROWS):
                            for u in range(PXW // 2):
                                idxvals[sl * 128 + k2 * 45 + e * 5 + u] = \
                                    rel0 + e * t["wrow"] + u
                        # W2 par matrices [45, 64] each
                        Wy = t["Wy"][a]                     # [9, 8]
                        Mx = t["Mx"][a]                     # [10, 8]
                        for par in range(2):
                            Mp = Mx[par::2]                 # [5, 8]
                            W2 = np.einsum("ei,uj->euij", Wy, Mp).reshape(45, 64)
                            iw[k2 * 45:(k2 + 1) * 45,
                               pidx * 256 + par * 128 + k2 * 64:
                               pidx * 256 + par * 128 + (k2 + 1) * 64] = \
                                W2.astype(np.float16)
            assert idxvals.max() < 32768
            cols = S * 8
            iidx[:, col0: col0 + cols] = _wrap_idx(idxvals)
            col0 += cols

        per_core.append(dict(ridx=ridx, rw=rw, rs=rs, iidx=iidx, iw=iw,
                             rmap=rmap, imap=imap))

    return dict(cat2=cat2, per_core=per_core, tabs=tabs,
                reg_batches=reg_batches, irr_batches=irr_batches,
                NRGtot=NRGtot, NPtot=NPtot, RCOLS=RCOLS, ICOLS=ICOLS,
                nunits_pad=nunits_pad, A=A)


def _stratum_gi0(reg_batches, r, q):
    for (rr, qq, G, gi0) in reg_batches:
        if (rr, qq) == (r, q):
            return gi0
    raise KeyError


def _irr_pi0(irr_batches, r):
    for (rr, S, pi0) in irr_batches:
        if rr == r:
            return pi0
    raise KeyError


def _wrap_idx(idxvals):
    """[N] linear idx values -> [128, N//16] int16 tile (16-partition wrap,
    replicated to all 128 partitions)."""
    n = len(idxvals)
    assert n % 16 == 0
    tile = np.zeros((128, n // 16), np.int16)
    lin = np.asarray(idxvals, np.int64)
    assert lin.min() >= 0 and lin.max() < 32768
    t16 = lin.reshape(n // 16, 16).T.astype(np.int16)   # [16, n/16]
    for rep in range(8):
        tile[rep * 16:(rep + 1) * 16, :] = t16
    return tile


# ----------------------------------------------------------------------------
# device program
# ----------------------------------------------------------------------------

def _build_program(prep):
    f16 = mybir.dt.float16
    f32 = mybir.dt.float32
    i16 = mybir.dt.int16
    COPY = mybir.ActivationFunctionType.Copy
    MULT = mybir.AluOpType.mult
    ADD = mybir.AluOpType.add
    NRGtot = prep["NRGtot"]
    NPtot = prep["NPtot"]
    RCOLS = prep["RCOLS"]
    ICOLS = prep["ICOLS"]
    NUP = prep["nunits_pad"]

    nc = bacc.Bacc("TRN2", num_swdge_queues=NQ)
    cat2_t = nc.dram_tensor("cat2", [NUP, UNIT], f16, kind="ExternalInput")
    ridx_t = nc.dram_tensor("ridx", [128, max(RCOLS, 1)], i16,
                            kind="ExternalInput")
    rw_t = nc.dram_tensor("rw", [128, max(NRGtot, 1) * 112], f16,
                          kind="ExternalInput")
    rs_t = nc.dram_tensor("rs", [128, max(NRGtot, 1)], f32,
                          kind="ExternalInput")
    iidx_t = nc.dram_tensor("iidx", [128, max(ICOLS, 1)], i16,
                            kind="ExternalInput")
    iw_t = nc.dram_tensor("iw", [128, max(NPtot, 1) * 256], f16,
                          kind="ExternalInput")
    oreg_t = nc.dram_tensor("out_reg", [112, max(NRGtot, 1) * 512], f16,
                            kind="ExternalOutput")
    oirr_t = nc.dram_tensor("out_irr", [128, max(NPtot, 1) * 64], f16,
                            kind="ExternalOutput")

    with TileContext(nc) as tc:
        with tc.tile_pool(name="const", bufs=1) as cpool, \
             tc.tile_pool(name="gat", bufs=5) as gpool, \
             tc.tile_pool(name="mv", bufs=6) as vpool, \
             tc.tile_pool(name="ps", bufs=4, space="PSUM") as pspool, \
             tc.tile_pool(name="ob", bufs=3) as obpool:

            ridx = cpool.tile([128, max(RCOLS, 1)], i16)
            rw = cpool.tile([128, max(NRGtot, 1) * 112], f16)
            rs = cpool.tile([128, max(NRGtot, 1)], f32)
            iidx = cpool.tile([128, max(ICOLS, 1)], i16)
            iw = cpool.tile([128, max(NPtot, 1) * 256], f16)
            nc.sync.dma_start(ridx[:, :], ridx_t[:, :])
            nc.sync.dma_start(rw[:, :], rw_t[:, :])
            nc.sync.dma_start(rs[:, :], rs_t[:, :])
            nc.sync.dma_start(iidx[:, :], iidx_t[:, :])
            nc.sync.dma_start(iw[:, :], iw_t[:, :])

            def region_ap(r, esize):
                return bass.AP(cat2_t, r * REGION * UNIT,
                               [(UNIT, 32768), (1, esize)])

            qrr = 0
            col0 = 0
            for (r, q, G, gi0) in prep["reg_batches"]:
                Gt = gpool.tile([128, G * ESIZE], f16, tag="gt")
                nc.gpsimd.dma_gather(
                    out_ap=Gt[:, :].rearrange("p (s e) -> p s e", e=ESIZE),
                    in_ap=region_ap(r, ESIZE),
                    idxs_ap=ridx[:, col0: col0 + G * 8],
                    num_idxs=G * 128, num_idxs_reg=G * 128,
                    elem_size=ESIZE, elem_step=UNIT,
                    queue_num=qrr % NQ)
                qrr += 1
                col0 += G * 8
                obuf = obpool.tile([112, G * 512], f16, tag="ob")
                for g in range(G):
                    gi = gi0 + g
                    base = g * ESIZE + 64 * q
                    t = vpool.tile([128, 512], f16, tag="t")
                    nc.vector.scalar_tensor_tensor(
                        out=t[:, :], in0=Gt[:, base + 64: base + 576],
                        scalar=rs[:, gi: gi + 1],
                        in1=Gt[:, base: base + 512],
                        op0=MULT, op1=ADD)
                    u = pspool.tile([112, 512], f32, tag="u")
                    nc.tensor.matmul(out=u[:, :],
                                     lhsT=rw[:, gi * 112: (gi + 1) * 112],
                                     rhs=t[:, :], start=True, stop=True)
                    nc.scalar.activation(obuf[:, g * 512: (g + 1) * 512],
                                         u[:, :], COPY)
                nc.sync.dma_start(oreg_t[:, gi0 * 512: (gi0 + G) * 512],
                                  obuf[:, :])

            col0 = 0
            for (r, S, pi0) in prep["irr_batches"]:
                Git = gpool.tile([128, S * UNIT], f16, tag="git")
                nc.gpsimd.dma_gather(
                    out_ap=Git[:, :].rearrange("p (s e) -> p s e", e=UNIT),
                    in_ap=region_ap(r, UNIT),
                    idxs_ap=iidx[:, col0: col0 + S * 8],
                    num_idxs=S * 128, num_idxs_reg=S * 128,
                    elem_size=UNIT, elem_step=UNIT,
                    queue_num=qrr % NQ)
                qrr += 1
                col0 += S * 8
                iob = obpool.tile([128, S * 64], f16, tag="iob")
                for sl in range(S):
                    pi = pi0 + sl
                    u2 = pspool.tile([128, 64], f32, tag="u2")
                    nc.tensor.matmul(
                        out=u2[:, :],
                        lhsT=iw[:, pi * 256: pi * 256 + 128],
                        rhs=Git[:, sl * UNIT: sl * UNIT + 64],
                        start=True, stop=False)
                    nc.tensor.matmul(
                        out=u2[:, :],
                        lhsT=iw[:, pi * 256 + 128: pi * 256 + 256],
                        rhs=Git[:, sl * UNIT + 64: sl * UNIT + 128],
                        start=False, stop=True)
                    nc.vector.tensor_copy(out=iob[:, sl * 64: (sl + 1) * 64],
                                          in_=u2[:, :])
                nc.sync.dma_start(oirr_t[:, pi0 * 64: (pi0 + S) * 64],
                                  iob[:, :])

    nc.finalize()
    return nc


# ----------------------------------------------------------------------------
# entry point
# ----------------------------------------------------------------------------

def kernel(f0, f1, f2, pixel, batch_index):
    global LAST_RESULTS
    prep = _host_prep(f0, f1, f2, pixel, batch_index)
    A = prep["A"]

    nc = _build_program(prep)

    in_maps = []
    for ccc in range(N_CORES):
        pc = prep["per_core"][ccc]
        in_maps.append({"cat2": prep["cat2"], "ridx": pc["ridx"],
                        "rw": pc["rw"], "rs": pc["rs"],
                        "iidx": pc["iidx"], "iw": pc["iw"]})

    res = run_bass_kernel_spmd(nc, in_maps, core_ids=list(range(N_CORES)),
                               trace=bool(os.environ.get("BASS_TRACE")))
    LAST_RESULTS = res

    out = np.zeros((A, 3, C, SIZE, SIZE), F32)
    NRGtot, NPtot = prep["NRGtot"], prep["NPtot"]
    for ccc in range(N_CORES):
        pc = prep["per_core"][ccc]
        raw = res.results[ccc]["out_reg"].astype(F32)
        # [112, NRG*512] -> [14jobs, 8i, NRG, 8j, 64c] -> [14, NRG, 64, 8, 8]
        rr = (raw.reshape(JOBS_PG, SIZE, NRGtot, SIZE, C)
              .transpose(0, 2, 4, 1, 3))
        if pc["rmap"]:
            gia = np.array([m[0] for m in pc["rmap"]])
            ka = np.array([m[1] for m in pc["rmap"]])
            sa = np.array([m[2] for m in pc["rmap"]])
            aa = np.array([m[3] for m in pc["rmap"]])
            out[aa, sa] = rr[ka, gia]
        if pc["imap"]:
            rawi = res.results[ccc]["out_irr"].astype(F32)
            # [128, NP*64]: part = k2*64 + i*8 + j -> [2, 8i, 8j, NP, 64c]
            ri = (rawi.reshape(2, SIZE, SIZE, NPtot, C)
                  .transpose(0, 3, 4, 1, 2))
            pia = np.array([m[0] for m in pc["imap"]])
            k2a = np.array([m[1] for m in pc["imap"]])
            sa = np.array([m[2] for m in pc["imap"]])
            aa = np.array([m[3] for m in pc["imap"]])
            out[aa, sa] = ri[k2a, pia]

    # zero x-invalid bins of regular jobs (irr path folds vx into Mx)
    for s in range(3):
        xm = prep["tabs"][s]["xmask"]          # [A, 8] bool
        if xm.any():
            aa, jj = np.nonzero(xm)
            out[aa, s, :, :, jj] = 0.0
    return out.reshape(A, 3 * C, SIZE, SIZE)


# ----------------------------------------------------------------------------
# numpy emulation of the device program (for offline validation)
# ----------------------------------------------------------------------------

def emulate(f0, f1, f2, pixel, batch_index):
    prep = _host_prep(f0, f1, f2, pixel, batch_index)
    A = prep["A"]
    cat2 = prep["cat2"]
    flat = cat2.reshape(-1)
    NRGtot, NPtot = prep["NRGtot"], prep["NPtot"]
    out = np.zeros((A, 3, C, SIZE, SIZE), F32)
    for ccc in range(N_CORES):
        pc = prep["per_core"][ccc]
        raw = np.zeros((112, max(NRGtot, 1) * 512), np.float16)
        col0 = 0
        for (r, q, G, gi0) in prep["reg_batches"]:
            # gather
            Gt = np.zeros((128, G * ESIZE), np.float16)
            for i in range(G * 128):
                p, sslot = i % 128, i // 128
                idx = int(pc["ridx"][i % 16, col0 + i // 16])
                st = (r * REGION + idx) * UNIT
                Gt[p, sslot * ESIZE: (sslot + 1) * ESIZE] = flat[st: st + ESIZE]
            col0 += G * 8
            for g in range(G):
                gi = gi0 + g
                base = g * ESIZE + 64 * q
                g32 = Gt.astype(F32)
                t = (g32[:, base + 64: base + 576]
                     * pc["rs"][:, gi: gi + 1]
                     + g32[:, base: base + 512]).astype(np.float16)
                u = (pc["rw"][:, gi * 112: (gi + 1) * 112].astype(F32).T
                     @ t.astype(F32))
                raw[:, gi * 512: (gi + 1) * 512] = u.astype(np.float16)
        rr = (raw.astype(F32).reshape(JOBS_PG, SIZE, NRGtot, SIZE, C)
              .transpose(0, 2, 4, 1, 3))
        if pc["rmap"]:
            gia = np.array([m[0] for m in pc["rmap"]])
            ka = np.array([m[1] for m in pc["rmap"]])
            sa = np.array([m[2] for m in pc["rmap"]])
            aa = np.array([m[3] for m in pc["rmap"]])
            out[aa, sa] = rr[ka, gia]

        rawi = np.zeros((128, max(NPtot, 1) * 64), np.float16)
        col0 = 0
        for (r, S, pi0) in prep["irr_batches"]:
            Git = np.zeros((128, S * UNIT), np.float16)
            for i in range(S * 128):
                p, sslot = i % 128, i // 128
                idx = int(pc["iidx"][i % 16, col0 + i // 16])
                st = (r * REGION + idx) * UNIT
                Git[p, sslot * UNIT: (sslot + 1) * UNIT] = flat[st: st + UNIT]
            col0 += S * 8
            for sl in range(S):
                pi = pi0 + sl
                u2 = (pc["iw"][:, pi * 256: pi * 256 + 128].astype(F32).T
                      @ Git[:, sl * UNIT: sl * UNIT + 64].astype(F32))
                u2 += (pc["iw"][:, pi * 256 + 128: pi * 256 + 256]
                       .astype(F32).T
                       @ Git[:, sl * UNIT + 64: sl * UNIT + 128].astype(F32))
                rawi[:, pi * 64: (pi + 1) * 64] = u2.astype(np.float16)
        if pc["imap"]:
            ri = (rawi.astype(F32).reshape(2, SIZE, SIZE, NPtot, C)
                  .transpose(0, 3, 4, 1, 2))
            pia = np.array([m[0] for m in pc["imap"]])
            k2a = np.array([m[1] for m in pc["imap"]])
            sa = np.array([m[2] for m in pc["imap"]])
            aa = np.array([m[3] for m in pc["imap"]])
            out[aa, sa] = ri[k2a, pia]

    for s in range(3):
        xm = prep["tabs"][s]["xmask"]
        if xm.any():
            aa, jj = np.nonzero(xm)
            out[aa, s, :, :, jj] = 0.0
    return out.reshape(A, 3 * C, SIZE, SIZE)


# revision 13
# speedup vs baseline: 1.5151x; 1.0049x over previous
"""Trainium2 Bass kernel for nn_CropperQAT (multi-scale RoIAlign with
fake-quantized rois) — v3.

Strategy (data-parallel over (roi, scale) jobs, 8 cores):
  * Host replicates the reference roi math bit-exactly (numpy f32), then for
    each job (roi a, scale s) derives a 9-row x 10-px fp16 feature window
    (all bilinear taps fit) plus interpolation weights.
  * Feature rows are RIGHT-PADDED by 8 replicated edge pixels so rois whose
    x taps clamp at the right edge stay on the regular (unit-stride) path;
    invalid (x > W) bins are zeroed during host unpack. Only left-clamped
    rois (x1 == 0 with roi_w < 8, non-unit stride) take the irregular path.
  * Features are stored channels-last fp16 as 2-px "units" (256B) so
    dma_gather (int16 indices, 256B-stride) can fetch windows; the unit
    space is split into regions so relative indices fit int16. Gathers are
    spread round-robin over 4 SWDGE queues so descriptor generation
    parallelizes across Q7 core pairs.
  * Device, x-regular jobs (partition = (job k, window row e), 14x9=126):
      - dma_gather batches up to 8 groups: [128, G*640] fp16
      - x-interp: ONE DVE scalar_tensor_tensor per group:
        t = G0 + r*G1 with r = fx/(1-fx); the (1-fx) factor is folded into
        the y-interp matrix Wy host-side.
      - y-interp: PE matmul with per-group block-diagonal [126->112]
        fp16 weights (handles y clamping/validity for free) -> PSUM f32
      - ACT copy PSUM -> fp16 output tile; batched DMA out.
  * x-irregular jobs (~4%): full bilinear as two accumulating matmuls
    (even/odd pixel parity) per 2-job pair slot; zero DVE work.
  * Host converts fp16 device output to f32 (fp16 path measures ~5e-4
    rel err; tolerance 2e-2).
"""
import os
import sys

sys.path.insert(0, "/opt/trn_rl_repo")

import ml_dtypes
import numpy as np

import concourse.bass as bass
import concourse.bacc as bacc
import concourse.mybir as mybir
from concourse.tile import TileContext
from concourse.bass_utils import run_bass_kernel_spmd

F32 = np.float32
BF16 = ml_dtypes.bfloat16
SIZE = 8
STRIDES = (4, 8, 16)
QS = np.float32(0.25)
C = 64
N_CORES = 8
JOBS_PG = 14                 # jobs per regular group (x9 rows = 126 partitions)
ROWS = 9                     # y window rows
PXW = 10                     # x window pixels (5 units)
XPAD = 8                     # replicated right-pad pixels per row
UNIT = 128                   # fp16 elems per unit (2 px * 64 ch)
ESIZE = PXW * C              # 640 fp16 per gathered row-window element
REGION = 30400               # units per region (int16 headroom: +1061 < 32768)
NGMAX = 8                    # groups per dma_gather batch
NSMAX = 8                    # irr pair-slots per dma_gather batch
NQ = 4                       # SWDGE queues for gather desc-gen parallelism

LAST_RESULTS = None


# ----------------------------------------------------------------------------
# host-side math (bit-exact replication of the jax reference)
# ----------------------------------------------------------------------------

def _fake_quant(x):
    return (np.clip(np.round(x / QS), -32768, 32767) * QS).astype(F32)


def _prep(c, L):
    valid = (c >= -1.0) & (c <= L)
    c = np.maximum(c, F32(0.0))
    low0 = np.floor(c).astype(np.int32)
    hi_edge = low0 >= L - 1
    low = np.where(hi_edge, L - 1, low0).astype(np.int32)
    high = np.where(hi_edge, L - 1, low0 + 1).astype(np.int32)
    c = np.where(hi_edge, F32(L - 1), c).astype(F32)
    frac = (c - low.astype(F32)).astype(F32)
    return low, high, frac, valid


def _scale_tables(pixel, batch_index, stride, H, W, base_px):
    """Per-job tables for one scale. base_px = pixel offset of this scale's
    image block in the channels-last concatenated (row-padded) tensor."""
    A = pixel.shape[0]
    Wp = W + XPAD
    st = F32(stride)
    half = F32(SIZE / 2.0)
    centers = (np.arange(SIZE, dtype=F32) + F32(0.5)).astype(F32)

    px = pixel[:, 0].astype(F32)
    py = pixel[:, 1].astype(F32)
    x1 = _fake_quant(np.maximum(px / st - half, F32(0.0)).astype(F32))
    y1 = _fake_quant(np.maximum(py / st - half, F32(0.0)).astype(F32))
    x2 = _fake_quant(np.maximum(px / st + half, F32(0.0)).astype(F32))
    y2 = _fake_quant(np.maximum(py / st + half, F32(0.0)).astype(F32))
    roi_w = np.maximum(x2 - x1, F32(1.0)).astype(F32)
    roi_h = np.maximum(y2 - y1, F32(1.0)).astype(F32)
    y = (y1[:, None] + centers[None, :] * (roi_h / F32(SIZE))[:, None]).astype(F32)
    x = (x1[:, None] + centers[None, :] * (roi_w / F32(SIZE))[:, None]).astype(F32)

    yl, yh, fy, vy = _prep(y, H)
    xl, xh, fx, vx = _prep(x, W)

    b = batch_index.astype(np.int64)

    # y window + Wy (y-interp matrix with validity folded in)
    wy0 = np.minimum(yl[:, 0], H - ROWS).astype(np.int64)
    ey_lo = yl.astype(np.int64) - wy0[:, None]
    ey_hi = yh.astype(np.int64) - wy0[:, None]
    assert ey_lo.min() >= 0 and ey_lo.max() <= 8
    assert ey_hi.min() >= 0 and ey_hi.max() <= 8
    vyf = vy.astype(F32)
    wl = ((F32(1.0) - fy) * vyf).astype(F32)
    wh = (fy * vyf).astype(F32)
    Wy = np.zeros((A, ROWS, SIZE), F32)
    aa = np.repeat(np.arange(A), SIZE)
    ii = np.tile(np.arange(SIZE), A)
    np.add.at(Wy, (aa, ey_lo.ravel(), ii), wl.ravel())
    np.add.at(Wy, (aa, ey_hi.ravel(), ii), wh.ravel())

    # --- regular-x classification against the PADDED row layout --------
    # virtual unit-stride taps a_j = xv0+j, b_j = a_j+1 into the padded row
    # (positions >= W-1 all hold F[W-1]); fx0 constant.
    x0f = (x1 + F32(0.5)).astype(F32)
    xv0 = np.floor(x0f).astype(np.int64)
    fx0 = (x0f - xv0.astype(F32)).astype(F32)
    jj8 = np.arange(SIZE, dtype=np.int64)
    a_j = xv0[:, None] + jj8[None, :]
    b_j = a_j + 1
    a_eff = np.minimum(a_j, W - 1)
    b_eff = np.minimum(b_j, W - 1)
    interior = (a_eff == xl) & (b_eff == xh) & (fx == fx0[:, None])
    hi_coll = (xl == W - 1) & (xh == W - 1) & (a_j >= W - 1)
    match = interior | hi_coll | (~vx)
    reg = ((roi_w == F32(8.0)) & np.all(match, axis=1)
           & (xv0 >= 0) & (xv0 + SIZE + 1 <= Wp - 1))
    # fold (1-fx0) into Wy for reg jobs; r = fx0/(1-fx0) as the stt scalar
    bx0 = (F32(1.0) - fx0).astype(F32)
    rsc = (fx0 / bx0).astype(F32)
    x0w = np.where(reg, xv0 & ~np.int64(1), 0)
    q0 = (xv0 - x0w).astype(np.int64)
    assert np.all(q0[reg] >= 0) and np.all(q0[reg] <= 1)
    assert np.all(x0w[reg] + PXW <= Wp)

    # host-side x-validity mask (reg jobs only; irr folds vx into Mx)
    xmask = (~vx) & reg[:, None]

    # irr: dense x matrix over the 10-px window at x0w=0 (validity folded)
    vxf = vx.astype(F32)
    Mx = np.zeros((A, PXW, SIZE), F32)
    irr_a = np.nonzero(~reg)[0]
    if irr_a.size:
        ex_lo = xl[irr_a].astype(np.int64)
        ex_hi = xh[irr_a].astype(np.int64)
        assert ex_lo.min() >= 0 and ex_lo.max() <= PXW - 1
        assert ex_hi.min() >= 0 and ex_hi.max() <= PXW - 1
        ai = np.repeat(irr_a, SIZE)
        ji = np.tile(np.arange(SIZE), irr_a.size)
        np.add.at(Mx, (ai, ex_lo.ravel(), ji),
                  ((F32(1.0) - fx[irr_a]) * vxf[irr_a]).ravel())
        np.add.at(Mx, (ai, ex_hi.ravel(), ji),
                  (fx[irr_a] * vxf[irr_a]).ravel())

    # window start unit per row e: u0 + e*(Wp//2)
    u0 = (base_px + (b * H + wy0) * Wp + x0w) // 2
    return dict(Wy=Wy, reg=reg, q0=q0, bx0=bx0, rsc=rsc, Mx=Mx,
                u0=u0.astype(np.int64), wrow=Wp // 2, xmask=xmask)


def _host_prep(f0, f1, f2, pixel, batch_index):
    A = pixel.shape[0]
    feats = (f0, f1, f2)

    # channels-last rows, right-padded with XPAD copies of the edge pixel
    blocks = []
    for f in feats:
        t = np.asarray(f, dtype=F32).transpose(0, 2, 3, 1)    # [N,H,W,C]
        pad = np.repeat(t[:, :, -1:, :], XPAD, axis=2)
        blocks.append(np.concatenate([t, pad], axis=2).reshape(-1, C))
    cat = np.concatenate(blocks, axis=0).astype(BF16)
    nunits = cat.shape[0] // 2
    nreg = (nunits - 1) // REGION + 1
    nunits_pad = (nreg - 1) * REGION + 32768 + 8
    cat2 = np.zeros((nunits_pad, UNIT), BF16)
    cat2[:nunits] = cat.reshape(nunits, UNIT)

    tabs = []
    base_px = 0
    for s, f in enumerate(feats):
        H, W = f.shape[2], f.shape[3]
        tabs.append(_scale_tables(np.asarray(pixel, F32),
                                  np.asarray(batch_index), STRIDES[s],
                                  H, W, base_px))
        base_px += 4 * H * (W + XPAD)

    # ---- flat job lists with stratum key (region, kind)
    jobs = []            # (s, a, region, parity, is_reg)
    for s in range(3):
        t = tabs[s]
        regions = t["u0"] // REGION
        for a in range(A):
            jobs.append((s, a, int(regions[a]), int(t["q0"][a]),
                         bool(t["reg"][a])))

    # sort jobs by window start unit for DRAM locality within gathers
    jobs.sort(key=lambda j: int(tabs[j[0]]["u0"][j[1]]))

    # stratified round-robin assignment:
    #   reg strata key (region, parity), irr strata key (region)
    core_reg = {}        # (region, parity) -> [list per core of (s,a)]
    core_irr = {}        # region -> [list per core]
    cnt_reg = {}
    cnt_irr = {}
    for (s, a, r, q, isreg) in jobs:
        if isreg:
            key = (r, q)
            lst = core_reg.setdefault(key, [[] for _ in range(N_CORES)])
            k = cnt_reg[key] = cnt_reg.get(key, 0) + 1
            lst[(k - 1) % N_CORES].append((s, a))
        else:
            lst = core_irr.setdefault(r, [[] for _ in range(N_CORES)])
            k = cnt_irr[r] = cnt_irr.get(r, 0) + 1
            lst[(k - 1) % N_CORES].append((s, a))

    # global group counts per stratum (max over cores)
    reg_strata = sorted(core_reg.keys())
    irr_strata = sorted(core_irr.keys())
    NB = {key: max(-(-len(lst) // JOBS_PG) for lst in core_reg[key])
          for key in reg_strata}
    NP_ = {r: max(-(-len(lst) // 2) for lst in core_irr[r])
           for r in irr_strata}
    NRGtot = sum(NB.values())
    NPtot = sum(NP_.values())

    # batches (static program structure)
    reg_batches = []     # (region, parity, n_groups_in_batch, gi0)
    gi = 0
    for (r, q) in reg_strata:
        nb = NB[(r, q)]
        while nb > 0:
            g = min(nb, NGMAX)
            reg_batches.append((r, q, g, gi))
            gi += g
            nb -= g
    irr_batches = []     # (region, n_slots_in_batch, pi0)
    pi = 0
    for r in irr_strata:
        npr = NP_[r]
        while npr > 0:
            sct = min(npr, NSMAX)
            irr_batches.append((r, sct, pi))
            pi += sct
            npr -= sct

    # ---- per-core packed device inputs + output mapping
    RCOLS = sum(8 * g for (_, _, g, _) in reg_batches)
    ICOLS = sum(8 * sct for (_, sct, _) in irr_batches)
    per_core = []
    for ccc in range(N_CORES):
        ridx = np.zeros((128, max(RCOLS, 1)), np.int16)
        rw = np.zeros((128, max(NRGtot, 1) * 112), BF16)
        rs = np.zeros((128, max(NRGtot, 1)), BF16)
        iidx = np.zeros((128, max(ICOLS, 1)), np.int16)
        iw = np.zeros((128, max(NPtot, 1) * 256), BF16)
        rmap = []        # (gi, k, s, a)
        imap = []        # (pi, k2, s, a)

        col0 = 0
        for (r, q, G, gi0) in reg_batches:
            lst = core_reg[(r, q)][ccc]
            idxvals = np.zeros((G * 128,), np.int64)
            sg0 = _stratum_gi0(reg_batches, r, q)
            for g in range(G):
                for k in range(JOBS_PG):
                    j = (gi0 - sg0 + g) * JOBS_PG + k
                    if j < len(lst):
                        s, a = lst[j]
                        t = tabs[s]
                        gidx = gi0 + g
                        rmap.append((gidx, k, s, a))
                        rel0 = int(t["u0"][a] - r * REGION)
                        for e in range(ROWS):
                            idxvals[g * 128 + k * ROWS + e] = rel0 + e * t["wrow"]
                        rw[k * ROWS:(k + 1) * ROWS,
                           gidx * 112 + k * SIZE: gidx * 112 + (k + 1) * SIZE] = \
                            (t["Wy"][a] * t["bx0"][a]).astype(BF16)
                        rs[k * ROWS:(k + 1) * ROWS, gidx] = t["rsc"][a]
            assert idxvals.max() < 32768
            cols = G * 8
            tilecols = _wrap_idx(idxvals)
            ridx[:, col0: col0 + cols] = tilecols
            col0 += cols

        col0 = 0
        for (r, S, pi0) in irr_batches:
            lst = core_irr[r][ccc]
            idxvals = np.zeros((S * 128,), np.int64)
            for sl in range(S):
                for k2 in range(2):
                    j = (pi0 - _irr_pi0(irr_batches, r) + sl) * 2 + k2
                    if j < len(lst):
                        s, a = lst[j]
                        t = tabs[s]
                        pidx = pi0 + sl
                        imap.append((pidx, k2, s, a))
                        rel0 = int(t["u0"][a] - r * REGION)
                        for e in range(ROWS):
                            for u in range(PXW // 2):
                                idxvals[sl * 128 + k2 * 45 + e * 5 + u] = \
                                    rel0 + e * t["wrow"] + u
                        # W2 par matrices [45, 64] each
                        Wy = t["Wy"][a]                     # [9, 8]
                        Mx = t["Mx"][a]                     # [10, 8]
                        for par in range(2):
                            Mp = Mx[par::2]                 # [5, 8]
                            W2 = np.einsum("ei,uj->euij", Wy, Mp).reshape(45, 64)
                            iw[k2 * 45:(k2 + 1) * 45,
                               pidx * 256 + par * 128 + k2 * 64:
                               pidx * 256 + par * 128 + (k2 + 1) * 64] = \
                                W2.astype(BF16)
            assert idxvals.max() < 32768
            cols = S * 8
            iidx[:, col0: col0 + cols] = _wrap_idx(idxvals)
            col0 += cols

        per_core.append(dict(ridx=ridx, rw=rw, rs=rs, iidx=iidx, iw=iw,
                             rmap=rmap, imap=imap))

    return dict(cat2=cat2, per_core=per_core, tabs=tabs,
                reg_batches=reg_batches, irr_batches=irr_batches,
                NRGtot=NRGtot, NPtot=NPtot, RCOLS=RCOLS, ICOLS=ICOLS,
                nunits_pad=nunits_pad, A=A)


def _stratum_gi0(reg_batches, r, q):
    for (rr, qq, G, gi0) in reg_batches:
        if (rr, qq) == (r, q):
            return gi0
    raise KeyError


def _irr_pi0(irr_batches, r):
    for (rr, S, pi0) in irr_batches:
        if rr == r:
            return pi0
    raise KeyError


def _wrap_idx(idxvals):
    """[N] linear idx values -> [128, N//16] int16 tile (16-partition wrap,
    replicated to all 128 partitions)."""
    n = len(idxvals)
    assert n % 16 == 0
    tile = np.zeros((128, n // 16), np.int16)
    lin = np.asarray(idxvals, np.int64)
    assert lin.min() >= 0 and lin.max() < 32768
    t16 = lin.reshape(n // 16, 16).T.astype(np.int16)   # [16, n/16]
    for rep in range(8):
        tile[rep * 16:(rep + 1) * 16, :] = t16
    return tile


# ----------------------------------------------------------------------------
# device program
# ----------------------------------------------------------------------------

def _build_program(prep):
    f16 = mybir.dt.float16
    bf16 = mybir.dt.bfloat16
    f32 = mybir.dt.float32
    i16 = mybir.dt.int16
    COPY = mybir.ActivationFunctionType.Copy
    MULT = mybir.AluOpType.mult
    ADD = mybir.AluOpType.add
    NRGtot = prep["NRGtot"]
    NPtot = prep["NPtot"]
    RCOLS = prep["RCOLS"]
    ICOLS = prep["ICOLS"]
    NUP = prep["nunits_pad"]

    nc = bacc.Bacc("TRN2", num_swdge_queues=NQ)
    cat2_t = nc.dram_tensor("cat2", [NUP, UNIT], bf16, kind="ExternalInput")
    ridx_t = nc.dram_tensor("ridx", [128, max(RCOLS, 1)], i16,
                            kind="ExternalInput")
    rw_t = nc.dram_tensor("rw", [128, max(NRGtot, 1) * 112], bf16,
                          kind="ExternalInput")
    rs_t = nc.dram_tensor("rs", [128, max(NRGtot, 1)], bf16,
                          kind="ExternalInput")
    iidx_t = nc.dram_tensor("iidx", [128, max(ICOLS, 1)], i16,
                            kind="ExternalInput")
    iw_t = nc.dram_tensor("iw", [128, max(NPtot, 1) * 256], bf16,
                          kind="ExternalInput")
    oreg_t = nc.dram_tensor("out_reg", [112, max(NRGtot, 1) * 512], f16,
                            kind="ExternalOutput")
    oirr_t = nc.dram_tensor("out_irr", [128, max(NPtot, 1) * 64], f16,
                            kind="ExternalOutput")

    with TileContext(nc) as tc:
        with tc.tile_pool(name="const", bufs=1) as cpool, \
             tc.tile_pool(name="gat", bufs=6) as gpool, \
             tc.tile_pool(name="mv", bufs=8) as vpool, \
             tc.tile_pool(name="ps", bufs=2, space="PSUM") as pspool, \
             tc.tile_pool(name="ob", bufs=3) as obpool:

            ridx = cpool.tile([128, max(RCOLS, 1)], i16)
            rw = cpool.tile([128, max(NRGtot, 1) * 112], bf16)
            rs = cpool.tile([128, max(NRGtot, 1)], bf16)
            iidx = cpool.tile([128, max(ICOLS, 1)], i16)
            iw = cpool.tile([128, max(NPtot, 1) * 256], bf16)
            nc.sync.dma_start(ridx[:, :], ridx_t[:, :])
            nc.sync.dma_start(rw[:, :], rw_t[:, :])
            nc.sync.dma_start(rs[:, :], rs_t[:, :])
            nc.sync.dma_start(iidx[:, :], iidx_t[:, :])
            nc.sync.dma_start(iw[:, :], iw_t[:, :])

            def region_ap(r, esize):
                return bass.AP(cat2_t, r * REGION * UNIT,
                               [(UNIT, 32768), (1, esize)])

            qrr = 0
            col0 = 0
            for (r, q, G, gi0) in prep["reg_batches"]:
                Gt = gpool.tile([128, G * ESIZE], bf16, tag="gt")
                nc.gpsimd.dma_gather(
                    out_ap=Gt[:, :].rearrange("p (s e) -> p s e", e=ESIZE),
                    in_ap=region_ap(r, ESIZE),
                    idxs_ap=ridx[:, col0: col0 + G * 8],
                    num_idxs=G * 128, num_idxs_reg=G * 128,
                    elem_size=ESIZE, elem_step=UNIT,
                    queue_num=qrr % NQ)
                qrr += 1
                col0 += G * 8
                obuf = obpool.tile([112, G * 512], f16, tag="ob")
                u = None
                for g in range(G):
                    gi = gi0 + g
                    base = g * ESIZE + 64 * q
                    t = vpool.tile([128, 512], bf16, tag="t")
                    nc.vector.scalar_tensor_tensor(
                        out=t[:, :], in0=Gt[:, base + 64: base + 576],
                        scalar=rs[:, gi: gi + 1],
                        in1=Gt[:, base: base + 512],
                        op0=MULT, op1=ADD)
                    if g % 2 == 0:
                        u = pspool.tile([112, 1024], f32, tag="u")
                    half = (g % 2) * 512
                    nc.tensor.matmul(out=u[:, half: half + 512],
                                     lhsT=rw[:, gi * 112: (gi + 1) * 112],
                                     rhs=t[:, :], start=True, stop=True)
                    if g % 2 == 1 or g == G - 1:
                        g0 = g - (g % 2)
                        w = (g - g0 + 1) * 512
                        nc.scalar.activation(
                            obuf[:, g0 * 512: g0 * 512 + w],
                            u[:, :w], COPY)
                nc.sync.dma_start(oreg_t[:, gi0 * 512: (gi0 + G) * 512],
                                  obuf[:, :])

            col0 = 0
            for (r, S, pi0) in prep["irr_batches"]:
                Git = gpool.tile([128, S * UNIT], bf16, tag="git")
                nc.gpsimd.dma_gather(
                    out_ap=Git[:, :].rearrange("p (s e) -> p s e", e=UNIT),
                    in_ap=region_ap(r, UNIT),
                    idxs_ap=iidx[:, col0: col0 + S * 8],
                    num_idxs=S * 128, num_idxs_reg=S * 128,
                    elem_size=UNIT, elem_step=UNIT,
                    queue_num=qrr % NQ)
                qrr += 1
                col0 += S * 8
                iob = obpool.tile([128, S * 64], f16, tag="iob")
                for sl in range(S):
                    pi = pi0 + sl
                    u2 = pspool.tile([128, 64], f32, tag="u2")
                    nc.tensor.matmul(
                        out=u2[:, :],
                        lhsT=iw[:, pi * 256: pi * 256 + 128],
                        rhs=Git[:, sl * UNIT: sl * UNIT + 64],
                        start=True, stop=False)
                    nc.tensor.matmul(
                        out=u2[:, :],
                        lhsT=iw[:, pi * 256 + 128: pi * 256 + 256],
                        rhs=Git[:, sl * UNIT + 64: sl * UNIT + 128],
                        start=False, stop=True)
                    nc.vector.tensor_copy(out=iob[:, sl * 64: (sl + 1) * 64],
                                          in_=u2[:, :])
                nc.sync.dma_start(oirr_t[:, pi0 * 64: (pi0 + S) * 64],
                                  iob[:, :])

    nc.finalize()
    return nc


# ----------------------------------------------------------------------------
# entry point
# ----------------------------------------------------------------------------

def kernel(f0, f1, f2, pixel, batch_index):
    global LAST_RESULTS
    prep = _host_prep(f0, f1, f2, pixel, batch_index)
    A = prep["A"]

    nc = _build_program(prep)

    in_maps = []
    for ccc in range(N_CORES):
        pc = prep["per_core"][ccc]
        in_maps.append({"cat2": prep["cat2"], "ridx": pc["ridx"],
                        "rw": pc["rw"], "rs": pc["rs"],
                        "iidx": pc["iidx"], "iw": pc["iw"]})

    res = run_bass_kernel_spmd(nc, in_maps, core_ids=list(range(N_CORES)),
                               trace=bool(os.environ.get("BASS_TRACE")))
    LAST_RESULTS = res

    out = np.zeros((A, 3, C, SIZE, SIZE), F32)
    NRGtot, NPtot = prep["NRGtot"], prep["NPtot"]
    for ccc in range(N_CORES):
        pc = prep["per_core"][ccc]
        raw = res.results[ccc]["out_reg"].astype(F32)
        # [112, NRG*512] -> [14jobs, 8i, NRG, 8j, 64c] -> [14, NRG, 64, 8, 8]
        rr = (raw.reshape(JOBS_PG, SIZE, NRGtot, SIZE, C)
              .transpose(0, 2, 4, 1, 3))
        if pc["rmap"]:
            gia = np.array([m[0] for m in pc["rmap"]])
            ka = np.array([m[1] for m in pc["rmap"]])
            sa = np.array([m[2] for m in pc["rmap"]])
            aa = np.array([m[3] for m in pc["rmap"]])
            out[aa, sa] = rr[ka, gia]
        if pc["imap"]:
            rawi = res.results[ccc]["out_irr"].astype(F32)
            # [128, NP*64]: part = k2*64 + i*8 + j -> [2, 8i, 8j, NP, 64c]
            ri = (rawi.reshape(2, SIZE, SIZE, NPtot, C)
                  .transpose(0, 3, 4, 1, 2))
            pia = np.array([m[0] for m in pc["imap"]])
            k2a = np.array([m[1] for m in pc["imap"]])
            sa = np.array([m[2] for m in pc["imap"]])
            aa = np.array([m[3] for m in pc["imap"]])
            out[aa, sa] = ri[k2a, pia]

    # zero x-invalid bins of regular jobs (irr path folds vx into Mx)
    for s in range(3):
        xm = prep["tabs"][s]["xmask"]          # [A, 8] bool
        if xm.any():
            aa, jj = np.nonzero(xm)
            out[aa, s, :, :, jj] = 0.0
    return out.reshape(A, 3 * C, SIZE, SIZE)


# ----------------------------------------------------------------------------
# numpy emulation of the device program (for offline validation)
# ----------------------------------------------------------------------------

def emulate(f0, f1, f2, pixel, batch_index):
    prep = _host_prep(f0, f1, f2, pixel, batch_index)
    A = prep["A"]
    cat2 = prep["cat2"]
    flat = cat2.reshape(-1)
    NRGtot, NPtot = prep["NRGtot"], prep["NPtot"]
    out = np.zeros((A, 3, C, SIZE, SIZE), F32)
    for ccc in range(N_CORES):
        pc = prep["per_core"][ccc]
        raw = np.zeros((112, max(NRGtot, 1) * 512), np.float16)
        col0 = 0
        for (r, q, G, gi0) in prep["reg_batches"]:
            # gather
            Gt = np.zeros((128, G * ESIZE), BF16)
            for i in range(G * 128):
                p, sslot = i % 128, i // 128
                idx = int(pc["ridx"][i % 16, col0 + i // 16])
                st = (r * REGION + idx) * UNIT
                Gt[p, sslot * ESIZE: (sslot + 1) * ESIZE] = flat[st: st + ESIZE]
            col0 += G * 8
            for g in range(G):
                gi = gi0 + g
                base = g * ESIZE + 64 * q
                g32 = Gt.astype(F32)
                t = (g32[:, base + 64: base + 576]
                     * pc["rs"][:, gi: gi + 1].astype(F32)
                     + g32[:, base: base + 512]).astype(BF16)
                u = (pc["rw"][:, gi * 112: (gi + 1) * 112].astype(F32).T
                     @ t.astype(F32))
                raw[:, gi * 512: (gi + 1) * 512] = u.astype(np.float16)
        rr = (raw.astype(F32).reshape(JOBS_PG, SIZE, NRGtot, SIZE, C)
              .transpose(0, 2, 4, 1, 3))
        if pc["rmap"]:
            gia = np.array([m[0] for m in pc["rmap"]])
            ka = np.array([m[1] for m in pc["rmap"]])
            sa = np.array([m[2] for m in pc["rmap"]])
            aa = np.array([m[3] for m in pc["rmap"]])
            out[aa, sa] = rr[ka, gia]

        rawi = np.zeros((128, max(NPtot, 1) * 64), np.float16)
        col0 = 0
        for (r, S, pi0) in prep["irr_batches"]:
            Git = np.zeros((128, S * UNIT), BF16)
            for i in range(S * 128):
                p, sslot = i % 128, i // 128
                idx = int(pc["iidx"][i % 16, col0 + i // 16])
                st = (r * REGION + idx) * UNIT
                Git[p, sslot * UNIT: (sslot + 1) * UNIT] = flat[st: st + UNIT]
            col0 += S * 8
            for sl in range(S):
                pi = pi0 + sl
                u2 = (pc["iw"][:, pi * 256: pi * 256 + 128].astype(F32).T
                      @ Git[:, sl * UNIT: sl * UNIT + 64].astype(F32))
                u2 += (pc["iw"][:, pi * 256 + 128: pi * 256 + 256]
                       .astype(F32).T
                       @ Git[:, sl * UNIT + 64: sl * UNIT + 128].astype(F32))
                rawi[:, pi * 64: (pi + 1) * 64] = u2.astype(np.float16)
        if pc["imap"]:
            ri = (rawi.astype(F32).reshape(2, SIZE, SIZE, NPtot, C)
                  .transpose(0, 3, 4, 1, 2))
            pia = np.array([m[0] for m in pc["imap"]])
            k2a = np.array([m[1] for m in pc["imap"]])
            sa = np.array([m[2] for m in pc["imap"]])
            aa = np.array([m[3] for m in pc["imap"]])
            out[aa, sa] = ri[k2a, pia]

    for s in range(3):
        xm = prep["tabs"][s]["xmask"]
        if xm.any():
            aa, jj = np.nonzero(xm)
            out[aa, s, :, :, jj] = 0.0
    return out.reshape(A, 3 * C, SIZE, SIZE)


# revision 22
# speedup vs baseline: 1.5574x; 1.0279x over previous
"""Trainium2 Bass kernel for nn_CropperQAT (multi-scale RoIAlign with
fake-quantized rois) — v3.

Strategy (data-parallel over (roi, scale) jobs, 8 cores):
  * Host replicates the reference roi math bit-exactly (numpy f32), then for
    each job (roi a, scale s) derives a 9-row x 10-px fp16 feature window
    (all bilinear taps fit) plus interpolation weights.
  * Feature rows are RIGHT-PADDED by 8 replicated edge pixels so rois whose
    x taps clamp at the right edge stay on the regular (unit-stride) path;
    invalid (x > W) bins are zeroed during host unpack. Only left-clamped
    rois (x1 == 0 with roi_w < 8, non-unit stride) take the irregular path.
  * Features are stored channels-last fp16 as 2-px "units" (256B) so
    dma_gather (int16 indices, 256B-stride) can fetch windows; the unit
    space is split into regions so relative indices fit int16. Gathers are
    spread round-robin over 4 SWDGE queues so descriptor generation
    parallelizes across Q7 core pairs.
  * Device, x-regular jobs (partition = (job k, window row e), 14x9=126):
      - dma_gather batches up to 8 groups: [128, G*640] fp16
      - x-interp: ONE DVE scalar_tensor_tensor per group:
        t = G0 + r*G1 with r = fx/(1-fx); the (1-fx) factor is folded into
        the y-interp matrix Wy host-side.
      - y-interp: PE matmul with per-group block-diagonal [126->112]
        fp16 weights (handles y clamping/validity for free) -> PSUM f32
      - ACT copy PSUM -> fp16 output tile; batched DMA out.
  * x-irregular jobs (~4%): full bilinear as two accumulating matmuls
    (even/odd pixel parity) per 2-job pair slot; zero DVE work.
  * Host converts fp16 device output to f32 (fp16 path measures ~5e-4
    rel err; tolerance 2e-2).
"""
import os
import sys

sys.path.insert(0, "/opt/trn_rl_repo")

import ml_dtypes
import numpy as np

import concourse.bass as bass
import concourse.bacc as bacc
import concourse.mybir as mybir
from concourse.tile import TileContext
from concourse.bass_utils import run_bass_kernel_spmd

F32 = np.float32
BF16 = ml_dtypes.bfloat16
SIZE = 8
STRIDES = (4, 8, 16)
QS = np.float32(0.25)
C = 64
N_CORES = 8
JOBS_PG = 14                 # jobs per regular group (x9 rows = 126 partitions)
ROWS = 9                     # y window rows
PXW = 10                     # x window pixels (5 units)
XPAD = 8                     # replicated right-pad pixels per row
UNIT = 128                   # fp16 elems per unit (2 px * 64 ch)
ESIZE = PXW * C              # 640 fp16 per gathered row-window element
REGION = 30400               # units per region (int16 headroom: +1061 < 32768)
NGMAX = 8                    # groups per dma_gather batch
NSMAX = 8                    # irr pair-slots per dma_gather batch
NQ = 4                       # SWDGE queues for gather desc-gen parallelism

LAST_RESULTS = None


# ----------------------------------------------------------------------------
# host-side math (bit-exact replication of the jax reference)
# ----------------------------------------------------------------------------

def _fake_quant(x):
    return (np.clip(np.round(x / QS), -32768, 32767) * QS).astype(F32)


def _prep(c, L):
    valid = (c >= -1.0) & (c <= L)
    c = np.maximum(c, F32(0.0))
    low0 = np.floor(c).astype(np.int32)
    hi_edge = low0 >= L - 1
    low = np.where(hi_edge, L - 1, low0).astype(np.int32)
    high = np.where(hi_edge, L - 1, low0 + 1).astype(np.int32)
    c = np.where(hi_edge, F32(L - 1), c).astype(F32)
    frac = (c - low.astype(F32)).astype(F32)
    return low, high, frac, valid


def _scale_tables(pixel, batch_index, stride, H, W, base_px):
    """Per-job tables for one scale. base_px = pixel offset of this scale's
    image block in the channels-last concatenated (row-padded) tensor."""
    A = pixel.shape[0]
    Wp = W + XPAD
    st = F32(stride)
    half = F32(SIZE / 2.0)
    centers = (np.arange(SIZE, dtype=F32) + F32(0.5)).astype(F32)

    px = pixel[:, 0].astype(F32)
    py = pixel[:, 1].astype(F32)
    x1 = _fake_quant(np.maximum(px / st - half, F32(0.0)).astype(F32))
    y1 = _fake_quant(np.maximum(py / st - half, F32(0.0)).astype(F32))
    x2 = _fake_quant(np.maximum(px / st + half, F32(0.0)).astype(F32))
    y2 = _fake_quant(np.maximum(py / st + half, F32(0.0)).astype(F32))
    roi_w = np.maximum(x2 - x1, F32(1.0)).astype(F32)
    roi_h = np.maximum(y2 - y1, F32(1.0)).astype(F32)
    y = (y1[:, None] + centers[None, :] * (roi_h / F32(SIZE))[:, None]).astype(F32)
    x = (x1[:, None] + centers[None, :] * (roi_w / F32(SIZE))[:, None]).astype(F32)

    yl, yh, fy, vy = _prep(y, H)
    xl, xh, fx, vx = _prep(x, W)

    b = batch_index.astype(np.int64)

    # y window + Wy (y-interp matrix with validity folded in)
    wy0 = np.minimum(yl[:, 0], H - ROWS).astype(np.int64)
    ey_lo = yl.astype(np.int64) - wy0[:, None]
    ey_hi = yh.astype(np.int64) - wy0[:, None]
    assert ey_lo.min() >= 0 and ey_lo.max() <= 8
    assert ey_hi.min() >= 0 and ey_hi.max() <= 8
    vyf = vy.astype(F32)
    wl = ((F32(1.0) - fy) * vyf).astype(F32)
    wh = (fy * vyf).astype(F32)
    Wy = np.zeros((A, ROWS, SIZE), F32)
    aa = np.repeat(np.arange(A), SIZE)
    ii = np.tile(np.arange(SIZE), A)
    np.add.at(Wy, (aa, ey_lo.ravel(), ii), wl.ravel())
    np.add.at(Wy, (aa, ey_hi.ravel(), ii), wh.ravel())

    # --- regular-x classification against the PADDED row layout --------
    # virtual unit-stride taps a_j = xv0+j, b_j = a_j+1 into the padded row
    # (positions >= W-1 all hold F[W-1]); fx0 constant.
    x0f = (x1 + F32(0.5)).astype(F32)
    xv0 = np.floor(x0f).astype(np.int64)
    fx0 = (x0f - xv0.astype(F32)).astype(F32)
    jj8 = np.arange(SIZE, dtype=np.int64)
    a_j = xv0[:, None] + jj8[None, :]
    b_j = a_j + 1
    a_eff = np.minimum(a_j, W - 1)
    b_eff = np.minimum(b_j, W - 1)
    interior = (a_eff == xl) & (b_eff == xh) & (fx == fx0[:, None])
    hi_coll = (xl == W - 1) & (xh == W - 1) & (a_j >= W - 1)
    match = interior | hi_coll | (~vx)
    reg = ((roi_w == F32(8.0)) & np.all(match, axis=1)
           & (xv0 >= 0) & (xv0 + SIZE + 1 <= Wp - 1))
    # fold (1-fx0) into Wy for reg jobs; r = fx0/(1-fx0) as the stt scalar
    bx0 = (F32(1.0) - fx0).astype(F32)
    rsc = (fx0 / bx0).astype(F32)
    x0w = np.where(reg, xv0 & ~np.int64(1), 0)
    q0 = (xv0 - x0w).astype(np.int64)
    assert np.all(q0[reg] >= 0) and np.all(q0[reg] <= 1)
    assert np.all(x0w[reg] + PXW <= Wp)

    # host-side x-validity mask (reg jobs only; irr folds vx into Mx)
    xmask = (~vx) & reg[:, None]

    # irr: dense x matrix over the 10-px window at x0w=0 (validity folded)
    vxf = vx.astype(F32)
    Mx = np.zeros((A, PXW, SIZE), F32)
    irr_a = np.nonzero(~reg)[0]
    if irr_a.size:
        ex_lo = xl[irr_a].astype(np.int64)
        ex_hi = xh[irr_a].astype(np.int64)
        assert ex_lo.min() >= 0 and ex_lo.max() <= PXW - 1
        assert ex_hi.min() >= 0 and ex_hi.max() <= PXW - 1
        ai = np.repeat(irr_a, SIZE)
        ji = np.tile(np.arange(SIZE), irr_a.size)
        np.add.at(Mx, (ai, ex_lo.ravel(), ji),
                  ((F32(1.0) - fx[irr_a]) * vxf[irr_a]).ravel())
        np.add.at(Mx, (ai, ex_hi.ravel(), ji),
                  (fx[irr_a] * vxf[irr_a]).ravel())

    # window start unit per row e: u0 + e*(Wp//2)
    u0 = (base_px + (b * H + wy0) * Wp + x0w) // 2
    return dict(Wy=Wy, reg=reg, q0=q0, bx0=bx0, rsc=rsc, Mx=Mx,
                u0=u0.astype(np.int64), wrow=Wp // 2, xmask=xmask)


def _host_prep(f0, f1, f2, pixel, batch_index):
    A = pixel.shape[0]
    feats = (f0, f1, f2)

    # channels-last rows, right-padded with XPAD copies of the edge pixel
    blocks = []
    for f in feats:
        t = np.asarray(f, dtype=F32).transpose(0, 2, 3, 1)    # [N,H,W,C]
        pad = np.repeat(t[:, :, -1:, :], XPAD, axis=2)
        blocks.append(np.concatenate([t, pad], axis=2).reshape(-1, C))
    cat = np.concatenate(blocks, axis=0).astype(BF16)
    nunits = cat.shape[0] // 2
    nreg = (nunits - 1) // REGION + 1
    nunits_pad = (nreg - 1) * REGION + 32768 + 8
    cat2 = np.zeros((nunits_pad, UNIT), BF16)
    cat2[:nunits] = cat.reshape(nunits, UNIT)

    tabs = []
    base_px = 0
    for s, f in enumerate(feats):
        H, W = f.shape[2], f.shape[3]
        tabs.append(_scale_tables(np.asarray(pixel, F32),
                                  np.asarray(batch_index), STRIDES[s],
                                  H, W, base_px))
        base_px += 4 * H * (W + XPAD)

    # ---- flat job lists with stratum key (region, kind)
    jobs = []            # (s, a, region, parity, is_reg)
    for s in range(3):
        t = tabs[s]
        regions = t["u0"] // REGION
        for a in range(A):
            jobs.append((s, a, int(regions[a]), int(t["q0"][a]),
                         bool(t["reg"][a])))

    # sort jobs by (fx0 bucket, window start unit): the bucket clusters
    # equal-r jobs into the same groups (enabling cheaper per-group x-interp
    # ops), u0 keeps DRAM locality within gathers
    def _jkey(j):
        s, a = j[0], j[1]
        return (int(round(float(tabs[s]["rsc"][a]) * 12)),
                int(tabs[s]["u0"][a]))
    jobs.sort(key=_jkey)

    # stratified round-robin assignment:
    #   reg strata key (region, parity), irr strata key (region)
    core_reg = {}        # (region, parity) -> [list per core of (s,a)]
    core_irr = {}        # region -> [list per core]
    cnt_reg = {}
    cnt_irr = {}
    for (s, a, r, q, isreg) in jobs:
        if isreg:
            key = (r, q)
            lst = core_reg.setdefault(key, [[] for _ in range(N_CORES)])
            k = cnt_reg[key] = cnt_reg.get(key, 0) + 1
            lst[(k - 1) % N_CORES].append((s, a))
        else:
            lst = core_irr.setdefault(r, [[] for _ in range(N_CORES)])
            k = cnt_irr[r] = cnt_irr.get(r, 0) + 1
            lst[(k - 1) % N_CORES].append((s, a))

    # global group counts per stratum (max over cores)
    reg_strata = sorted(core_reg.keys())
    irr_strata = sorted(core_irr.keys())
    NB = {key: max(-(-len(lst) // JOBS_PG) for lst in core_reg[key])
          for key in reg_strata}
    NP_ = {r: max(-(-len(lst) // 2) for lst in core_irr[r])
           for r in irr_strata}
    NRGtot = sum(NB.values())
    NPtot = sum(NP_.values())

    # batches (static program structure)
    reg_batches = []     # (region, parity, n_groups_in_batch, gi0)
    gi = 0
    for (r, q) in reg_strata:
        nb = NB[(r, q)]
        while nb > 0:
            g = min(nb, NGMAX)
            reg_batches.append((r, q, g, gi))
            gi += g
            nb -= g
    irr_batches = []     # (region, n_slots_in_batch, pi0)
    pi = 0
    for r in irr_strata:
        npr = NP_[r]
        while npr > 0:
            sct = min(npr, NSMAX)
            irr_batches.append((r, sct, pi))
            pi += sct
            npr -= sct

    # ---- per-core packed device inputs + output mapping
    RCOLS = sum(8 * g for (_, _, g, _) in reg_batches)
    ICOLS = sum(8 * sct for (_, sct, _) in irr_batches)
    per_core = []
    gvals = {}           # gi -> set of r values (across all cores)
    for ccc in range(N_CORES):
        ridx = np.zeros((128, max(RCOLS, 1)), np.int16)
        rw = np.zeros((126, max(NRGtot, 1) * 8), BF16)
        rs = np.zeros((128, max(NRGtot, 1)), BF16)
        iidx = np.zeros((128, max(ICOLS, 1)), np.int16)
        iw = np.zeros((90, max(NPtot, 1) * 128), BF16)
        rmap = []        # (gi, k, s, a)
        imap = []        # (pi, k2, s, a)

        col0 = 0
        for (r, q, G, gi0) in reg_batches:
            lst = core_reg[(r, q)][ccc]
            idxvals = np.zeros((G * 128,), np.int64)
            sg0 = _stratum_gi0(reg_batches, r, q)
            for g in range(G):
                for k in range(JOBS_PG):
                    j = (gi0 - sg0 + g) * JOBS_PG + k
                    if j < len(lst):
                        s, a = lst[j]
                        t = tabs[s]
                        gidx = gi0 + g
                        rmap.append((gidx, k, s, a))
                        rel0 = int(t["u0"][a] - r * REGION)
                        for e in range(ROWS):
                            idxvals[g * 128 + k * ROWS + e] = rel0 + e * t["wrow"]
                        rw[k * ROWS:(k + 1) * ROWS,
                           gidx * SIZE: (gidx + 1) * SIZE] = \
                            (t["Wy"][a] * t["bx0"][a]).astype(BF16)
                        rs[k * ROWS:(k + 1) * ROWS, gidx] = t["rsc"][a]
                        gvals.setdefault(gidx, set()).add(float(t["rsc"][a]))
            assert idxvals.max() < 32768
            cols = G * 8
            tilecols = _wrap_idx(idxvals)
            ridx[:, col0: col0 + cols] = tilecols
            col0 += cols

        col0 = 0
        for (r, S, pi0) in irr_batches:
            lst = core_irr[r][ccc]
            idxvals = np.zeros((S * 128,), np.int64)
            for sl in range(S):
                for k2 in range(2):
                    j = (pi0 - _irr_pi0(irr_batches, r) + sl) * 2 + k2
                    if j < len(lst):
                        s, a = lst[j]
                        t = tabs[s]
                        pidx = pi0 + sl
                        imap.append((pidx, k2, s, a))
                        rel0 = int(t["u0"][a] - r * REGION)
                        for e in range(ROWS):
                            for u in range(PXW // 2):
                                idxvals[sl * 128 + k2 * 45 + e * 5 + u] = \
                                    rel0 + e * t["wrow"] + u
                        # W2 par matrices [45, 64] each
                        Wy = t["Wy"][a]                     # [9, 8]
                        Mx = t["Mx"][a]                     # [10, 8]
                        for par in range(2):
                            Mp = Mx[par::2]                 # [5, 8]
                            W2 = np.einsum("ei,uj->euij", Wy, Mp).reshape(45, 64)
                            iw[k2 * 45:(k2 + 1) * 45,
                               pidx * 128 + par * 64:
                               pidx * 128 + (par + 1) * 64] = \
                                W2.astype(BF16)
            assert idxvals.max() < 32768
            cols = S * 8
            iidx[:, col0: col0 + cols] = _wrap_idx(idxvals)
            col0 += cols

        per_core.append(dict(ridx=ridx, rw=rw, rs=rs, iidx=iidx, iw=iw,
                             rmap=rmap, imap=imap))

    # per-group x-interp op (must be uniform across SPMD cores):
    # 0 = all r==0 (skip blend), 1 = all r==1 (tensor_add), 2 = general stt
    gflags = []
    for gi in range(max(NRGtot, 1)):
        vals = gvals.get(gi, set())
        if not vals or vals == {0.0}:
            gflags.append(0)
        elif vals == {1.0}:
            gflags.append(1)
        else:
            gflags.append(2)

    return dict(cat2=cat2, per_core=per_core, tabs=tabs,
                reg_batches=reg_batches, irr_batches=irr_batches,
                NRGtot=NRGtot, NPtot=NPtot, RCOLS=RCOLS, ICOLS=ICOLS,
                nunits_pad=nunits_pad, A=A, gflags=gflags)


def _stratum_gi0(reg_batches, r, q):
    for (rr, qq, G, gi0) in reg_batches:
        if (rr, qq) == (r, q):
            return gi0
    raise KeyError


def _irr_pi0(irr_batches, r):
    for (rr, S, pi0) in irr_batches:
        if rr == r:
            return pi0
    raise KeyError


def _wrap_idx(idxvals):
    """[N] linear idx values -> [128, N//16] int16 tile (16-partition wrap,
    replicated to all 128 partitions)."""
    n = len(idxvals)
    assert n % 16 == 0
    tile = np.zeros((128, n // 16), np.int16)
    lin = np.asarray(idxvals, np.int64)
    assert lin.min() >= 0 and lin.max() < 32768
    t16 = lin.reshape(n // 16, 16).T.astype(np.int16)   # [16, n/16]
    for rep in range(8):
        tile[rep * 16:(rep + 1) * 16, :] = t16
    return tile


# ----------------------------------------------------------------------------
# device program
# ----------------------------------------------------------------------------

def _build_program(prep):
    f16 = mybir.dt.float16
    bf16 = mybir.dt.bfloat16
    f32 = mybir.dt.float32
    i16 = mybir.dt.int16
    COPY = mybir.ActivationFunctionType.Copy
    MULT = mybir.AluOpType.mult
    ADD = mybir.AluOpType.add
    NRGtot = prep["NRGtot"]
    NPtot = prep["NPtot"]
    RCOLS = prep["RCOLS"]
    ICOLS = prep["ICOLS"]
    NUP = prep["nunits_pad"]

    nc = bacc.Bacc("TRN2", num_swdge_queues=NQ)
    cat2_t = nc.dram_tensor("cat2", [NUP, UNIT], bf16, kind="ExternalInput")
    ridx_t = nc.dram_tensor("ridx", [128, max(RCOLS, 1)], i16,
                            kind="ExternalInput")
    rw_t = nc.dram_tensor("rw", [126, max(NRGtot, 1) * 8], bf16,
                          kind="ExternalInput")
    rs_t = nc.dram_tensor("rs", [128, max(NRGtot, 1)], bf16,
                          kind="ExternalInput")
    iidx_t = nc.dram_tensor("iidx", [128, max(ICOLS, 1)], i16,
                            kind="ExternalInput")
    iw_t = nc.dram_tensor("iw", [90, max(NPtot, 1) * 128], bf16,
                          kind="ExternalInput")
    oreg_t = nc.dram_tensor("out_reg", [112, max(NRGtot, 1) * 512], f16,
                            kind="ExternalOutput")
    oirr_t = nc.dram_tensor("out_irr", [128, max(NPtot, 1) * 64], f16,
                            kind="ExternalOutput")

    with TileContext(nc) as tc:
        with tc.tile_pool(name="const", bufs=1) as cpool, \
             tc.tile_pool(name="gat", bufs=6) as gpool, \
             tc.tile_pool(name="mv", bufs=8) as vpool, \
             tc.tile_pool(name="ps", bufs=2, space="PSUM") as pspool, \
             tc.tile_pool(name="ob", bufs=3) as obpool:

            ridx = cpool.tile([128, max(RCOLS, 1)], i16)
            # +112 col pad so the per-job strided expansion APs stay in-bounds
            rw = cpool.tile([128, max(NRGtot, 1) * 112 + 112], bf16)
            rs = cpool.tile([128, max(NRGtot, 1)], bf16)
            iidx = cpool.tile([128, max(ICOLS, 1)], i16)
            iw = cpool.tile([128, max(NPtot, 1) * 256 + 256], bf16)
            nc.sync.dma_start(ridx[:, :], ridx_t[:, :])
            nc.sync.dma_start(rs[:, :], rs_t[:, :])
            nc.sync.dma_start(iidx[:, :], iidx_t[:, :])
            # block-diagonal weight tiles: zero-fill once, then land each
            # job-block via a strided DMA from the dense DRAM copy
            nc.vector.memset(rw[:, :], 0.0)
            nc.vector.memset(iw[:, :], 0.0)
            NRG1 = max(NRGtot, 1)
            NP1 = max(NPtot, 1)
            for k in range(JOBS_PG):
                out_ap = rw[k * ROWS:(k + 1) * ROWS,
                            k * 8: k * 8 + NRG1 * 112] \
                    .rearrange("p (g c) -> p g c", c=112)[:, :, 0:8]
                in_ap = rw_t[k * ROWS:(k + 1) * ROWS, :] \
                    .rearrange("p (g i) -> p g i", i=8)
                nc.sync.dma_start(out_ap, in_ap)
            for k2 in range(2):
                out_ap = iw[k2 * 45:(k2 + 1) * 45,
                            k2 * 64: k2 * 64 + NP1 * 256] \
                    .rearrange("p (s q c) -> p s q c", q=2, c=128)[:, :, :, 0:64]
                in_ap = iw_t[k2 * 45:(k2 + 1) * 45, :] \
                    .rearrange("p (s q c) -> p s q c", q=2, c=64)
                nc.sync.dma_start(out_ap, in_ap)

            def region_ap(r, esize):
                return bass.AP(cat2_t, r * REGION * UNIT,
                               [(UNIT, 32768), (1, esize)])

            qrr = 0
            col0 = 0
            for (r, q, G, gi0) in prep["reg_batches"]:
                Gt = gpool.tile([128, G * ESIZE], bf16, tag="gt")
                nc.gpsimd.dma_gather(
                    out_ap=Gt[:, :].rearrange("p (s e) -> p s e", e=ESIZE),
                    in_ap=region_ap(r, ESIZE),
                    idxs_ap=ridx[:, col0: col0 + G * 8],
                    num_idxs=G * 128, num_idxs_reg=G * 128,
                    elem_size=ESIZE, elem_step=UNIT,
                    queue_num=qrr % NQ)
                qrr += 1
                col0 += G * 8
                obuf = obpool.tile([112, G * 512], f16, tag="ob")
                u = None
                for g in range(G):
                    gi = gi0 + g
                    base = g * ESIZE + 64 * q
                    flag = prep["gflags"][gi]
                    if flag == 0:
                        rhs = Gt[:, base: base + 512]
                    else:
                        t = vpool.tile([128, 512], bf16, tag="t")
                        if flag == 1:
                            nc.vector.tensor_add(
                                t[:, :], Gt[:, base: base + 512],
                                Gt[:, base + 64: base + 576])
                        else:
                            nc.vector.scalar_tensor_tensor(
                                out=t[:, :], in0=Gt[:, base + 64: base + 576],
                                scalar=rs[:, gi: gi + 1],
                                in1=Gt[:, base: base + 512],
                                op0=MULT, op1=ADD)
                        rhs = t[:, :]
                    if g % 2 == 0:
                        u = pspool.tile([112, 1024], f32, tag="u")
                    half = (g % 2) * 512
                    nc.tensor.matmul(out=u[:, half: half + 512],
                                     lhsT=rw[:, gi * 112: (gi + 1) * 112],
                                     rhs=rhs, start=True, stop=True)
                    if g % 2 == 1 or g == G - 1:
                        g0 = g - (g % 2)
                        w = (g - g0 + 1) * 512
                        nc.scalar.activation(
                            obuf[:, g0 * 512: g0 * 512 + w],
                            u[:, :w], COPY)
                nc.sync.dma_start(oreg_t[:, gi0 * 512: (gi0 + G) * 512],
                                  obuf[:, :])

            col0 = 0
            for (r, S, pi0) in prep["irr_batches"]:
                Git = gpool.tile([128, S * UNIT], bf16, tag="git")
                nc.gpsimd.dma_gather(
                    out_ap=Git[:, :].rearrange("p (s e) -> p s e", e=UNIT),
                    in_ap=region_ap(r, UNIT),
                    idxs_ap=iidx[:, col0: col0 + S * 8],
                    num_idxs=S * 128, num_idxs_reg=S * 128,
                    elem_size=UNIT, elem_step=UNIT,
                    queue_num=qrr % NQ)
                qrr += 1
                col0 += S * 8
                iob = obpool.tile([128, S * 64], f16, tag="iob")
                for sl in range(S):
                    pi = pi0 + sl
                    u2 = pspool.tile([128, 64], f32, tag="u2")
                    nc.tensor.matmul(
                        out=u2[:, :],
                        lhsT=iw[:, pi * 256: pi * 256 + 128],
                        rhs=Git[:, sl * UNIT: sl * UNIT + 64],
                        start=True, stop=False)
                    nc.tensor.matmul(
                        out=u2[:, :],
                        lhsT=iw[:, pi * 256 + 128: pi * 256 + 256],
                        rhs=Git[:, sl * UNIT + 64: sl * UNIT + 128],
                        start=False, stop=True)
                    nc.vector.tensor_copy(out=iob[:, sl * 64: (sl + 1) * 64],
                                          in_=u2[:, :])
                nc.sync.dma_start(oirr_t[:, pi0 * 64: (pi0 + S) * 64],
                                  iob[:, :])

    nc.finalize()
    return nc


# ----------------------------------------------------------------------------
# entry point
# ----------------------------------------------------------------------------

def kernel(f0, f1, f2, pixel, batch_index):
    global LAST_RESULTS
    prep = _host_prep(f0, f1, f2, pixel, batch_index)
    A = prep["A"]

    nc = _build_program(prep)

    in_maps = []
    for ccc in range(N_CORES):
        pc = prep["per_core"][ccc]
        in_maps.append({"cat2": prep["cat2"], "ridx": pc["ridx"],
                        "rw": pc["rw"], "rs": pc["rs"],
                        "iidx": pc["iidx"], "iw": pc["iw"]})

    res = run_bass_kernel_spmd(nc, in_maps, core_ids=list(range(N_CORES)),
                               trace=bool(os.environ.get("BASS_TRACE")))
    LAST_RESULTS = res

    out = np.zeros((A, 3, C, SIZE, SIZE), F32)
    NRGtot, NPtot = prep["NRGtot"], prep["NPtot"]
    for ccc in range(N_CORES):
        pc = prep["per_core"][ccc]
        raw = res.results[ccc]["out_reg"].astype(F32)
        # [112, NRG*512] -> [14jobs, 8i, NRG, 8j, 64c] -> [14, NRG, 64, 8, 8]
        rr = (raw.reshape(JOBS_PG, SIZE, NRGtot, SIZE, C)
              .transpose(0, 2, 4, 1, 3))
        if pc["rmap"]:
            gia = np.array([m[0] for m in pc["rmap"]])
            ka = np.array([m[1] for m in pc["rmap"]])
            sa = np.array([m[2] for m in pc["rmap"]])
            aa = np.array([m[3] for m in pc["rmap"]])
            out[aa, sa] = rr[ka, gia]
        if pc["imap"]:
            rawi = res.results[ccc]["out_irr"].astype(F32)
            # [128, NP*64]: part = k2*64 + i*8 + j -> [2, 8i, 8j, NP, 64c]
            ri = (rawi.reshape(2, SIZE, SIZE, NPtot, C)
                  .transpose(0, 3, 4, 1, 2))
            pia = np.array([m[0] for m in pc["imap"]])
            k2a = np.array([m[1] for m in pc["imap"]])
            sa = np.array([m[2] for m in pc["imap"]])
            aa = np.array([m[3] for m in pc["imap"]])
            out[aa, sa] = ri[k2a, pia]

    # zero x-invalid bins of regular jobs (irr path folds vx into Mx)
    for s in range(3):
        xm = prep["tabs"][s]["xmask"]          # [A, 8] bool
        if xm.any():
            aa, jj = np.nonzero(xm)
            out[aa, s, :, :, jj] = 0.0
    return out.reshape(A, 3 * C, SIZE, SIZE)


# ----------------------------------------------------------------------------
# numpy emulation of the device program (for offline validation)
# ----------------------------------------------------------------------------

def emulate(f0, f1, f2, pixel, batch_index):
    prep = _host_prep(f0, f1, f2, pixel, batch_index)
    A = prep["A"]
    cat2 = prep["cat2"]
    flat = cat2.reshape(-1)
    NRGtot, NPtot = prep["NRGtot"], prep["NPtot"]
    out = np.zeros((A, 3, C, SIZE, SIZE), F32)
    for ccc in range(N_CORES):
        pc = prep["per_core"][ccc]
        raw = np.zeros((112, max(NRGtot, 1) * 512), np.float16)
        col0 = 0
        for (r, q, G, gi0) in prep["reg_batches"]:
            # gather
            Gt = np.zeros((128, G * ESIZE), BF16)
            for i in range(G * 128):
                p, sslot = i % 128, i // 128
                idx = int(pc["ridx"][i % 16, col0 + i // 16])
                st = (r * REGION + idx) * UNIT
                Gt[p, sslot * ESIZE: (sslot + 1) * ESIZE] = flat[st: st + ESIZE]
            col0 += G * 8
            for g in range(G):
                gi = gi0 + g
                base = g * ESIZE + 64 * q
                g32 = Gt.astype(F32)
                t = (g32[:, base + 64: base + 576]
                     * pc["rs"][:, gi: gi + 1].astype(F32)
                     + g32[:, base: base + 512]).astype(BF16)
                W = np.zeros((126, 112), F32)
                for k in range(JOBS_PG):
                    W[k * ROWS:(k + 1) * ROWS, k * SIZE:(k + 1) * SIZE] = \
                        pc["rw"][k * ROWS:(k + 1) * ROWS,
                                 gi * SIZE:(gi + 1) * SIZE].astype(F32)
                u = W.T @ t[:126].astype(F32)
                raw[:, gi * 512: (gi + 1) * 512] = u.astype(np.float16)
        rr = (raw.astype(F32).reshape(JOBS_PG, SIZE, NRGtot, SIZE, C)
              .transpose(0, 2, 4, 1, 3))
        if pc["rmap"]:
            gia = np.array([m[0] for m in pc["rmap"]])
            ka = np.array([m[1] for m in pc["rmap"]])
            sa = np.array([m[2] for m in pc["rmap"]])
            aa = np.array([m[3] for m in pc["rmap"]])
            out[aa, sa] = rr[ka, gia]

        rawi = np.zeros((128, max(NPtot, 1) * 64), np.float16)
        col0 = 0
        for (r, S, pi0) in prep["irr_batches"]:
            Git = np.zeros((128, S * UNIT), BF16)
            for i in range(S * 128):
                p, sslot = i % 128, i // 128
                idx = int(pc["iidx"][i % 16, col0 + i // 16])
                st = (r * REGION + idx) * UNIT
                Git[p, sslot * UNIT: (sslot + 1) * UNIT] = flat[st: st + UNIT]
            col0 += S * 8
            for sl in range(S):
                pi = pi0 + sl
                u2 = np.zeros((128, 64), F32)
                for par in range(2):
                    X = np.zeros((128, 128), F32)
                    for k2 in range(2):
                        X[k2 * 45:(k2 + 1) * 45, k2 * 64:(k2 + 1) * 64] = \
                            pc["iw"][k2 * 45:(k2 + 1) * 45,
                                     pi * 128 + par * 64:
                                     pi * 128 + (par + 1) * 64].astype(F32)
                    u2 += (X.T @ Git[:, sl * UNIT + par * 64:
                                     sl * UNIT + (par + 1) * 64].astype(F32))
                rawi[:, pi * 64: (pi + 1) * 64] = u2.astype(np.float16)
        if pc["imap"]:
            ri = (rawi.astype(F32).reshape(2, SIZE, SIZE, NPtot, C)
                  .transpose(0, 3, 4, 1, 2))
            pia = np.array([m[0] for m in pc["imap"]])
            k2a = np.array([m[1] for m in pc["imap"]])
            sa = np.array([m[2] for m in pc["imap"]])
            aa = np.array([m[3] for m in pc["imap"]])
            out[aa, sa] = ri[k2a, pia]

    for s in range(3):
        xm = prep["tabs"][s]["xmask"]
        if xm.any():
            aa, jj = np.nonzero(xm)
            out[aa, s, :, :, jj] = 0.0
    return out.reshape(A, 3 * C, SIZE, SIZE)
